# revision 30
# baseline (speedup 1.0000x reference)
"""3-layer GAT on 8 trn2 NeuronCores — dst-aligned gather design.

Strategy (graph/data parallel):
  - Nodes sorted by (grid in-degree, forced-lo count) and chunked into
    392 blocks of 128 dst slots; position j (0..48) holds 8 consecutive
    blocks, one per core, so all cores share one compiled program with
    identical per-position grid shapes.
  - Per layer: each core transforms its shard with rhs = [W | W@as | W@ad]
    (bf16), writes table rows [h(Fo) | as(heads)] (bf16, 768B rows),
    AllGather (Shared output) -> full table everywhere.
  - Aggregation: per dst block, edges are laid out in a [128 dst-slot,
    T cols] grid — the dma_gather's natural placement (row i -> partition
    i%128) puts every edge in its destination's partition. Softmax and the
    weighted feature sum then reduce along the free axis on DVE/ACT only:
    no scatter matmuls.  Self-loops occupy col 0 via a direct DMA from the
    core's own shard (no descriptors).  int16 gather indices are handled
    with two windows (rows [0,32768) and [17408,50176)); per-node
    forced-lo/forced-hi splits are balanced via the overlap band.
    Grid holes gather a harmless row and are killed by a 0/1 mask on s.
  - Layer 2 output is mask-summed per core; mean + linear head on host.
"""

import os
import numpy as np
import ml_dtypes

# ---------------- problem constants (must match reference) ----------------
N = 50000
E = 800000
IN_C = 128
HID = 64
HEADS = 4
OUT_C = 64
F1 = HEADS * HID  # 256

# ---------------- sharding geometry ----------------
NCORES = 8
NPOS = 49         # dst block positions per core
BS = 128          # dst slots per block
NPC = NPOS * BS   # 6272 rows per core
RTOT = NCORES * NPC  # 50176 table rows
LO_LIM = 32768    # lo window rows [0, LO_LIM)
HI_BASE = 17408   # hi window rows [HI_BASE, HI_BASE+32768)
MID_BASE = 8704   # mid window rows [MID_BASE, MID_BASE+32768)
WBASE = {"lo": 0, "mid": MID_BASE, "hi": HI_BASE}
EL = 384          # table elems/row layers 0/1 (256 h + 4 as + pad), bf16
EL2 = 128         # table elems/row layer 2 (64 h + 1 as + pad)
CH = 16           # gather/compute chunk columns
LAST_POS_NODES = 106   # nodes per block at position 48 (22 pads each)

BF16 = ml_dtypes.bfloat16


# ---------------- host preprocessing ----------------

def preprocess(edge_index):
    """Node->(core,pos,slot) assignment and per-core gather grid arrays.

    Returns dict with:
      row:    [N] table row of each node
      xperm:  [RTOT] node id at each table row (-1 pads)
      shapes: list of NPOS (LC, HC) shared across cores
      chunks: list per pos of chunk descriptors
              (seg 'lo' includes self col 0)
      idx16:  [NCORES][128, TOTC] int16 gather indices per core
      mask01: [NCORES][128, TOTM] f32 1/0 slot validity per chunk col
      maskc:  [NCORES][128, NPOS] f32 1.0 for real dst slots
      totc, totm: widths
    """
    src = np.asarray(edge_index[0], np.int64)
    dst = np.asarray(edge_index[1], np.int64)
    deg = np.bincount(dst, minlength=N)  # grid degree (self col excluded)

    # pass 1: provisional rows by degree
    def assign_rows(order):
        # position p takes nodes order[cnt : cnt+8*sz] -> 8 blocks
        row = np.full(N, -1, np.int64)
        xperm = np.full(RTOT, -1, np.int64)
        cnt = 0
        for p in range(NPOS):
            sz = BS if p < NPOS - 1 else LAST_POS_NODES
            for c in range(NCORES):
                nodes = order[cnt:cnt + sz]
                cnt += sz
                base = c * NPC + p * BS
                # real nodes at slots [BS-sz, BS): keeps slot 0 a pad
                # at position 48 on every core.
                slots = np.arange(BS - sz, BS)
                row[nodes] = base + slots
                xperm[base + slots] = nodes
        assert cnt == N
        return row, xperm

    # band counts per dst node given rows:
    # B1 [0,MID_BASE) lo-only; B2 [MID_BASE,HI_BASE) lo/mid;
    # B3 [HI_BASE,LO_LIM) any; B4 [LO_LIM,MID_BASE+32768) mid/hi;
    # B5 [MID_BASE+32768,RTOT) hi-only
    MID_LIM = MID_BASE + 32768

    def band_counts(rowv):
        er = rowv[src]
        a = np.bincount(dst[er < MID_BASE], minlength=N)
        ab = np.bincount(dst[er < HI_BASE], minlength=N)
        gh = np.bincount(dst[er >= LO_LIM], minlength=N)
        h = np.bincount(dst[er >= MID_LIM], minlength=N)
        return a, ab, gh, h

    def sort_key(a, ab, gh, h):
        k = deg.astype(np.int64)
        for v in (a, h, ab, gh):
            k = k * 64 + np.minimum(v, 63)
        return k

    order1 = np.argsort(-deg, kind="stable")
    row1, _ = assign_rows(order1)
    k1 = sort_key(*band_counts(row1))
    order2 = np.argsort(-k1, kind="stable")
    row2, _ = assign_rows(order2)
    k2 = sort_key(*band_counts(row2))
    order3 = np.argsort(-k2, kind="stable")
    row, xperm = assign_rows(order3)

    erow = row[src]
    eslot = row[dst]  # dst table row -> (core, pos, slot)

    # per-node edge lists
    eo = np.argsort(eslot, kind="stable")
    erow_s = erow[eo]
    eslot_s = eslot[eo]
    bounds = np.searchsorted(eslot_s, np.arange(RTOT + 1))

    nd = np.zeros((RTOT, 6), np.int64)  # d, a, ab, gh, h per table row
    for r in range(RTOT):
        rr = erow_s[bounds[r]:bounds[r + 1]]
        nd[r, 0] = len(rr)
        nd[r, 1] = int((rr < MID_BASE).sum())
        nd[r, 2] = int((rr < HI_BASE).sum())
        nd[r, 3] = int((rr >= LO_LIM).sum())
        nd[r, 4] = int((rr >= MID_LIM).sum())

    # shared (LC, MC, HC) per position (Hall conditions + small scan)
    shapes = []
    for p in range(NPOS):
        rows_p = np.concatenate([np.arange(c * NPC + p * BS,
                                           c * NPC + (p + 1) * BS)
                                 for c in range(NCORES)])
        d = nd[rows_p, 0]
        a = nd[rows_p, 1]
        ab = nd[rows_p, 2]
        gh = nd[rows_p, 3]
        h = nd[rows_p, 4]
        ma, mab, mgh, mh, md = (int(a.max()), int(ab.max()), int(gh.max()),
                                int(h.max()), int(d.max()))
        best = None
        for LC in range(ma, ma + 8):
            for HC in range(mh, mh + 8):
                MC = max(0, mab - LC, mgh - HC, md - LC - HC)
                T = LC + MC + HC
                sp = max(LC, MC, HC)
                if best is None or (T, sp) < (best[0], best[1]):
                    best = (T, sp, LC, MC, HC)
        shapes.append((best[2], best[3], best[4]))

    # chunk layout per position: seg lo = [self] + LC cols, then mid, hi
    chunks = []   # per pos: list of (kind, gcol0, ncols, ng, idx_off, m_off)
    totc = 0      # idx cols (int16), per gather chunk: 8 * ng
    totm = 0      # mask cols
    for p in range(NPOS):
        LC, MC, HC = shapes[p]
        cl = []
        c0 = 0
        while c0 < 1 + LC:
            nc_ = min(CH, 1 + LC - c0)
            ng = nc_ - 1 if c0 == 0 else nc_   # gathered cols (col0 self)
            cl.append(("lo", c0, nc_, ng, totc, totm))
            totc += 8 * ng
            totm += nc_
            c0 += nc_
        for kind, KC, base in (("mid", MC, 1 + LC), ("hi", HC, 1 + LC + MC)):
            c0 = 0
            while c0 < KC:
                nc_ = min(CH, KC - c0)
                cl.append((kind, base + c0, nc_, nc_, totc, totm))
                totc += 8 * nc_
                totm += nc_
                c0 += nc_
        chunks.append(cl)

    # dummy row for mid-grid holes: must be valid in both windows.
    DUMMY = 2 * NPC + 48 * BS + 0  # core 2, pos 48, slot 0 (a pad row)
    assert HI_BASE <= DUMMY < LO_LIM and xperm[DUMMY] == -1

    idx16 = [np.zeros((128, totc), np.int16) for _ in range(NCORES)]
    mask01 = [np.zeros((128, totm), np.float32) for _ in range(NCORES)]
    mask4 = [np.zeros((128, totm * 4), np.float32) for _ in range(NCORES)]
    maskc = [np.zeros((128, NPOS), np.float32) for _ in range(NCORES)]

    for c in range(NCORES):
        for p in range(NPOS):
            LC, MC, HC = shapes[p]
            base_row = c * NPC + p * BS
            # grid values: grid[s, col] = table row or -1 hole
            grid = np.full((BS, 1 + LC + MC + HC), -1, np.int64)
            for s in range(BS):
                r = base_row + s
                rr = erow_s[bounds[r]:bounds[r + 1]]
                b1 = rr[rr < MID_BASE]
                b2 = rr[(rr >= MID_BASE) & (rr < HI_BASE)]
                b3 = rr[(rr >= HI_BASE) & (rr < LO_LIM)]
                b4 = rr[(rr >= LO_LIM) & (rr < MID_LIM)]
                b5 = rr[rr >= MID_LIM]
                lo = list(b1)
                hi = list(b5)
                mid = []
                t2 = min(LC - len(lo), len(b2))
                assert t2 >= 0
                lo += list(b2[:t2]); mid += list(b2[t2:])
                t4 = min(HC - len(hi), len(b4))
                assert t4 >= 0
                hi += list(b4[:t4]); mid += list(b4[t4:])
                t3l = min(LC - len(lo), len(b3))
                lo += list(b3[:t3l])
                t3h = min(HC - len(hi), len(b3) - t3l)
                hi += list(b3[t3l:t3l + t3h])
                mid += list(b3[t3l + t3h:])
                assert (len(lo) <= LC and len(mid) <= MC
                        and len(hi) <= HC), (c, p, s)
                grid[s, 1:1 + len(lo)] = lo
                grid[s, 1 + LC:1 + LC + len(mid)] = mid
                grid[s, 1 + LC + MC:1 + LC + MC + len(hi)] = hi
                if xperm[r] >= 0:
                    maskc[c][s, p] = 1.0
                    grid[s, 0] = base_row + s  # self (marker only)

            for (kind, g0, nc_, ng, ioff, moff) in chunks[p]:
                # mask
                m = (grid[:, g0:g0 + nc_] >= 0).astype(np.float32)
                mask01[c][:, moff:moff + nc_] = m
                mask4[c][:, 4 * moff:4 * (moff + nc_)] = np.repeat(m, 4,
                                                                   axis=1)
                # gather idx for gathered cols
                gc0 = g0 + (1 if g0 == 0 else 0)
                sub = grid[:, gc0:gc0 + ng]          # [128, ng]
                base = WBASE[kind]
                vals = np.where(sub >= 0, sub - base, DUMMY - base)
                # positions i = col*128 + slot (holes gather the dummy row;
                # trailing -1 trim caused device hangs on all-dummy chunks)
                flat = vals.T.reshape(-1)            # [ng*128]
                assert flat.max(initial=0) < 32768 and flat.min(initial=0) >= 0
                K = ng * 128
                w = flat.reshape(K // 16, 16).T.astype(np.int16)  # [16,K/16]
                idx16[c][:, ioff:ioff + 8 * ng] = np.tile(w, (8, 1))

    return dict(row=row, xperm=xperm, shapes=shapes, chunks=chunks,
                idx16=idx16, mask01=mask01, mask4=mask4, maskc=maskc,
                totc=totc, totm=totm, deg=deg)


def host_weights(inputs):
    """Extended weight matrices with folded attention vectors (bf16)."""
    def ext(W, a_s, a_d, heads):
        Wh = W.reshape(W.shape[0], heads, HID)
        Was = np.einsum("khc,hc->kh", Wh, a_s)
        Wad = np.einsum("khc,hc->kh", Wh, a_d)
        return np.concatenate([W, Was, Wad], axis=1).astype(np.float32)

    W0e = ext(np.asarray(inputs["W0"], np.float32),
              np.asarray(inputs["a0s"], np.float32),
              np.asarray(inputs["a0d"], np.float32), HEADS)      # [128, 264]
    W1e = ext(np.asarray(inputs["W1"], np.float32),
              np.asarray(inputs["a1s"], np.float32),
              np.asarray(inputs["a1d"], np.float32), HEADS)      # [256, 264]
    W2e = ext(np.asarray(inputs["W2"], np.float32),
              np.asarray(inputs["a2s"], np.float32),
              np.asarray(inputs["a2d"], np.float32), 1)          # [256, 66]
    return W0e, W1e, W2e


def build_core_inputs(inputs, pp):
    x = np.asarray(inputs["x"], np.float32)
    W0e, W1e, W2e = host_weights(inputs)

    consts = dict(
        w0e=W0e.astype(BF16),
        w1e=W1e.reshape(2, 128, F1 + 2 * HEADS).astype(BF16),
        w2e=W2e.reshape(2, 128, HID + 2).astype(BF16),
        b0r=np.tile(np.asarray(inputs["b0"], np.float32), (128, 1)),
        b1r=np.tile(np.asarray(inputs["b1"], np.float32), (128, 1)),
        b2r=np.tile(np.asarray(inputs["b2"], np.float32), (128, 1)),
        ident=np.eye(128, dtype=np.float32).astype(BF16),
    )

    in_maps = []
    for c in range(NCORES):
        xtb = np.zeros((NPOS, IN_C, BS), np.float32)
        rows = np.arange(c * NPC, (c + 1) * NPC)
        nodes = pp["xperm"][rows].reshape(NPOS, BS)
        for b in range(NPOS):
            nb = nodes[b]
            valid = nb >= 0
            if valid.any():
                xtb[b][:, valid] = x[nb[valid]].T
        m = dict(
            xtb=xtb.astype(BF16),
            idx16=pp["idx16"][c],
            mask01=pp["mask01"][c].astype(BF16),
            mask4=pp["mask4"][c].astype(BF16),
            maskc=pp["maskc"][c],
            **consts,
        )
        in_maps.append(m)
    return in_maps


# ---------------- numpy emulation of the device data path ----------------

def emulate(inputs, pp=None):
    """Emulate the device path (fp32 math, bf16 rounding on tables)."""
    if pp is None:
        pp = preprocess(np.asarray(inputs["edge_index"]))
    x = np.asarray(inputs["x"], np.float32)
    W0e, W1e, W2e = host_weights(inputs)
    h = np.zeros((RTOT, IN_C), np.float32)
    valid = pp["xperm"] >= 0
    h[valid] = x[pp["xperm"][valid]]
    h = h.astype(BF16).astype(np.float32)

    biases = [np.asarray(inputs["b0"], np.float32),
              np.asarray(inputs["b1"], np.float32),
              np.asarray(inputs["b2"], np.float32)]
    Ws = [W0e, W1e, W2e]
    heads_l = [HEADS, HEADS, 1]
    Fo_l = [F1, F1, HID]

    shapes = pp["shapes"]
    chunks = pp["chunks"]

    for layer in range(3):
        We = Ws[layer].astype(BF16).astype(np.float32)
        heads = heads_l[layer]
        Fo = Fo_l[layer]
        tb = h @ We                       # [RTOT, Fo+2*heads]
        table = tb[:, :Fo + heads].astype(BF16).astype(np.float32)
        ad_all = tb[:, Fo + heads:Fo + 2 * heads]  # fp32 on device
        out = np.zeros((RTOT, Fo), np.float32)
        for c in range(NCORES):
            for p in range(NPOS):
                LC, MC, HC = shapes[p]
                base_row = c * NPC + p * BS
                agg = np.zeros((BS, Fo), np.float32)
                den = np.zeros((BS, heads), np.float32)
                for (kind, g0, nc_, ng, ioff, moff) in chunks[p]:
                    gcols = nc_
                    g = np.zeros((BS, gcols, Fo + heads), np.float32)
                    if g0 == 0:
                        g[:, 0, :] = table[base_row:base_row + BS]
                    gc_off = 1 if g0 == 0 else 0
                    if ng > 0:
                        w = pp["idx16"][c][:16, ioff:ioff + 8 * ng]
                        flat = w.T.reshape(-1).astype(np.int64)
                        base = WBASE[kind]
                        rows = np.where(flat >= 0, flat + base, 0)
                        gat = table[rows].reshape(ng, BS, Fo + heads)
                        g[:, gc_off:gc_off + ng, :] = gat.transpose(1, 0, 2)
                    m = pp["mask01"][c][:, moff:moff + nc_].astype(
                        BF16).astype(np.float32)
                    a_s = g[:, :, Fo:Fo + heads]
                    ad = ad_all[base_row:base_row + BS][:, None, :]
                    z = a_s + ad
                    lr = np.maximum(z, 0.2 * z)
                    s = np.exp(lr).astype(BF16).astype(np.float32)
                    s = s * m[:, :, None]
                    s = s.astype(BF16).astype(np.float32)
                    hfeat = g[:, :, :Fo].reshape(BS, gcols, heads, HID)
                    tmp = (hfeat * s[:, :, :, None]).astype(BF16).astype(
                        np.float32)
                    agg += tmp.sum(axis=1).reshape(BS, Fo)
                    den += s.sum(axis=1)
                o = agg.reshape(BS, heads, HID) / (den + 1e-16)[:, :, None]
                o = o.reshape(BS, Fo) + biases[layer]
                if layer < 2:
                    o = np.maximum(o, 0.0)
                out[base_row:base_row + BS] = o
        h = out.astype(BF16).astype(np.float32) if layer < 2 else out

    g = h[valid].sum(axis=0, keepdims=True) / N
    return (g @ np.asarray(inputs["hw"], np.float32)
            + np.asarray(inputs["hb"], np.float32)).astype(np.float32)


# ---------------- device kernel ----------------

def build_kernel(pp):
    import concourse.bacc as bacc
    import concourse.mybir as mybir
    import concourse.tile as tile
    from concourse import library_config

    f32 = mybir.dt.float32
    bf16 = mybir.dt.bfloat16
    i16 = mybir.dt.int16
    Alu = mybir.AluOpType
    Act = mybir.ActivationFunctionType
    Ax = mybir.AxisListType

    shapes = pp["shapes"]
    chunks = pp["chunks"]
    totc = pp["totc"]
    totm = pp["totm"]

    nc = bacc.Bacc("TRN2", target_bir_lowering=False, debug=False,
                   num_devices=NCORES, num_swdge_queues=4)

    # ---- I/O ----
    xtb_d = nc.dram_tensor("xtb", [NPOS, IN_C, BS], bf16, kind="ExternalInput")
    idx16_d = nc.dram_tensor("idx16", [128, totc], i16, kind="ExternalInput")
    mask01_d = nc.dram_tensor("mask01", [128, totm], bf16,
                              kind="ExternalInput")
    mask4_d = nc.dram_tensor("mask4", [128, totm * 4], bf16,
                             kind="ExternalInput")
    maskc_d = nc.dram_tensor("maskc", [128, NPOS], f32, kind="ExternalInput")
    w0e_d = nc.dram_tensor("w0e", [IN_C, F1 + 2 * HEADS], bf16,
                           kind="ExternalInput")
    w1e_d = nc.dram_tensor("w1e", [2, 128, F1 + 2 * HEADS], bf16,
                           kind="ExternalInput")
    w2e_d = nc.dram_tensor("w2e", [2, 128, HID + 2], bf16,
                           kind="ExternalInput")
    b0r_d = nc.dram_tensor("b0r", [128, F1], f32, kind="ExternalInput")
    b1r_d = nc.dram_tensor("b1r", [128, F1], f32, kind="ExternalInput")
    b2r_d = nc.dram_tensor("b2r", [128, HID], f32, kind="ExternalInput")
    ident_d = nc.dram_tensor("ident", [128, 128], bf16, kind="ExternalInput")
    out_d = nc.dram_tensor("out_part", [1, OUT_C], f32, kind="ExternalOutput")

    shared = os.environ.get("GAT_SHARED", "1") == "1"
    kw = dict(addr_space="Shared") if shared else {}
    shard01 = nc.dram_tensor("shard01", [NPC, EL], bf16)
    table01 = nc.dram_tensor("table01", [RTOT, EL], bf16, **kw)
    shard2 = nc.dram_tensor("shard2", [NPC, EL2], bf16)
    table2 = nc.dram_tensor("table2", [RTOT, EL2], bf16, **kw)

    rg = [list(range(NCORES))]

    with tile.TileContext(nc) as tc:
        with (
            tc.tile_pool(name="const", bufs=1) as cpool,
            tc.tile_pool(name="big", bufs=1) as bigpool,
            tc.tile_pool(name="work", bufs=3) as wpool,
            tc.tile_pool(name="gather", bufs=4) as gpool,
            tc.tile_pool(name="small", bufs=4) as spool,
            tc.tile_pool(name="att", bufs=3) as apool,
            tc.tile_pool(name="psum", bufs=2, space="PSUM") as ppool,
            tc.tile_pool(name="psum1", bufs=1, space="PSUM") as ppool1,
        ):
            def load_const(tag, dram, shape, dtype=f32, view=None):
                t = cpool.tile(shape, dtype, tag=tag)
                nc.sync.dma_start(out=t[:], in_=view if view is not None
                                  else dram[:])
                return t

            w0e_s = load_const("w0e", w0e_d, [IN_C, F1 + 2 * HEADS], bf16)
            w1e_s = load_const("w1e", w1e_d, [128, 2, F1 + 2 * HEADS], bf16,
                               view=w1e_d[:].rearrange("c p j -> p c j"))
            w2e_s = load_const("w2e", w2e_d, [128, 2, HID + 2], bf16,
                               view=w2e_d[:].rearrange("c p j -> p c j"))
            b0r_s = load_const("b0r", b0r_d, [128, F1])
            b1r_s = load_const("b1r", b1r_d, [128, F1])
            b2r_s = load_const("b2r", b2r_d, [128, HID])
            ident_s = load_const("ident", ident_d, [128, 128], bf16)
            identf_s = cpool.tile([128, 128], f32, tag="identf")
            nc.vector.tensor_copy(out=identf_s[:], in_=ident_s[:])
            idx16_s = load_const("idx16", idx16_d, [128, totc], i16)
            mask01_s = load_const("mask01", mask01_d, [128, totm], bf16)
            mask4_s = load_const("mask4", mask4_d, [128, totm * 4], bf16)
            maskc_s = load_const("maskc", maskc_d, [128, NPOS])

            nc.gpsimd.load_library(library_config.mlp)

            hT = bigpool.tile([128, 2, NPC], bf16, tag="hT")
            ad_all = bigpool.tile([128, NPOS * HEADS], f32, tag="ad_all")

            def transform(layer):
                heads = 1 if layer == 2 else HEADS
                Fo = HID if layer == 2 else F1
                ncols = Fo + 2 * heads
                el = EL2 if layer == 2 else EL
                shard = shard2 if layer == 2 else shard01
                for b in range(NPOS):
                    ps = ppool.tile([128, 512], f32, tag="tps", space="PSUM")
                    if layer == 0:
                        xb = wpool.tile([IN_C, BS], bf16, tag="xtb")
                        nc.sync.dma_start(out=xb[:], in_=xtb_d[b])
                        nc.tensor.matmul(out=ps[:, :ncols], lhsT=xb[:],
                                         rhs=w0e_s[:], start=True, stop=True)
                    else:
                        we = w1e_s if layer == 1 else w2e_s
                        for k2 in range(2):
                            nc.tensor.matmul(
                                out=ps[:, :ncols],
                                lhsT=hT[:, k2, b * BS:(b + 1) * BS],
                                rhs=we[:, k2, :],
                                start=(k2 == 0), stop=(k2 == 1))
                    tb = wpool.tile([128, el], bf16, tag="tbout")
                    nc.vector.tensor_copy(out=tb[:, :Fo + heads],
                                          in_=ps[:, :Fo + heads])
                    nc.vector.tensor_copy(
                        out=ad_all[:, b * heads:(b + 1) * heads],
                        in_=ps[:, Fo + heads:Fo + 2 * heads])
                    nc.sync.dma_start(out=shard[b * BS:(b + 1) * BS, :],
                                      in_=tb[:])

            def allgather(layer):
                shard = shard2 if layer == 2 else shard01
                table = table2 if layer == 2 else table01
                nc.gpsimd.collective_compute(
                    "AllGather", mybir.AluOpType.bypass,
                    replica_groups=rg, ins=[shard[:].opt()],
                    outs=[table[:].opt()])

            def aggregate(layer):
                heads = 1 if layer == 2 else HEADS
                Fo = HID if layer == 2 else F1
                el = EL2 if layer == 2 else EL
                shard = shard2 if layer == 2 else shard01
                table = table2 if layer == 2 else table01
                brep = (b0r_s if layer == 0 else
                        (b1r_s if layer == 1 else b2r_s))
                views = {"lo": table[0:LO_LIM, :],
                         "mid": table[MID_BASE:MID_BASE + 32768, :],
                         "hi": table[HI_BASE:HI_BASE + 32768, :]}
                if layer == 2:
                    psum_sum = ppool1.tile([1, OUT_C], f32, tag="sum",
                                           space="PSUM")
                sub = int(os.environ.get("GAT_SUB", "99"))
                qn = [0]
                for b in range(NPOS):
                    agg = spool.tile([128, Fo], f32, tag="agg")
                    den = spool.tile([128, heads], f32, tag="den")
                    first = True
                    for (kind, g0, nc_, ng, ioff, moff) in chunks[b]:
                        g = gpool.tile([128, CH, el], bf16, tag="g")
                        if g0 == 0:
                            nc.sync.dma_start(
                                out=g[:, 0:1, :],
                                in_=shard[b * BS:(b + 1) * BS, :]
                                .unsqueeze(1))
                        gc_off = 1 if g0 == 0 else 0
                        if ng > 0:
                            nc.gpsimd.dma_gather(
                                g[:, gc_off:gc_off + ng, :], views[kind],
                                idx16_s[:, ioff:ioff + 8 * ng],
                                128 * ng, 128 * ng, el,
                                single_packet=False,
                                queue_num=qn[0] % 4)
                            qn[0] += 1
                        if sub < 2:
                            continue
                        # z = as + ad  (prefix-contiguous out; strided in0)
                        z = apool.tile([128, CH, heads], f32, tag="z")
                        z2d = z[:].rearrange("p t h -> p (t h)")
                        nc.vector.tensor_tensor(
                            out=z[:, :nc_, :],
                            in0=g[:, :nc_, Fo:Fo + heads],
                            in1=ad_all[:, b * heads:(b + 1) * heads]
                            .unsqueeze(1).broadcast_to([128, nc_, heads]),
                            op=Alu.add)
                        nh = nc_ * heads
                        z2 = apool.tile([128, CH * heads], f32, tag="z2")
                        nc.vector.tensor_scalar(out=z2[:, :nh],
                                                in0=z2d[:, :nh],
                                                scalar1=0.2, scalar2=None,
                                                op0=Alu.mult)
                        lr = apool.tile([128, CH * heads], f32, tag="lr")
                        nc.vector.tensor_tensor(out=lr[:, :nh],
                                                in0=z2d[:, :nh],
                                                in1=z2[:, :nh],
                                                op=Alu.max)
                        s_t = apool.tile([128, CH * heads], bf16, tag="s")
                        nc.scalar.activation(s_t[:, :nh], lr[:, :nh],
                                             Act.Exp)
                        sm = apool.tile([128, CH, heads], bf16, tag="sm")
                        sm2d = sm[:].rearrange("p t h -> p (t h)")
                        msrc = mask4_s if heads == HEADS else mask01_s
                        nc.vector.tensor_tensor(
                            out=sm2d[:, :nh],
                            in0=s_t[:, :nh],
                            in1=msrc[:, heads * moff:heads * moff + nh],
                            op=Alu.mult)
                        if sub < 3:
                            continue
                        # tmp[t, f] = g[t, f] * sm[t, head(f)]  (t-major)
                        tmp = wpool.tile([128, CH, Fo], bf16, tag="tmp")
                        for hh in range(heads):
                            nc.vector.tensor_tensor(
                                out=tmp[:, :nc_, hh * HID:(hh + 1) * HID],
                                in0=g[:, :nc_, hh * HID:(hh + 1) * HID],
                                in1=sm[:, :nc_, hh:hh + 1]
                                .broadcast_to([128, nc_, HID]),
                                op=Alu.mult)
                        if sub < 4:
                            continue
                        # pairwise-fold reductions over t (prefix 2D slices)
                        tmp2 = tmp[:].rearrange("p t f -> p (t f)")
                        k = nc_
                        while k > 1:
                            pr = k // 2
                            nc.vector.tensor_tensor(
                                out=tmp2[:, :pr * Fo],
                                in0=tmp2[:, :pr * Fo],
                                in1=tmp2[:, (k - pr) * Fo:k * Fo],
                                op=Alu.add)
                            k -= pr
                        k = nc_
                        while k > 1:
                            pr = k // 2
                            nc.vector.tensor_tensor(
                                out=sm2d[:, :pr * heads],
                                in0=sm2d[:, :pr * heads],
                                in1=sm2d[:, (k - pr) * heads:k * heads],
                                op=Alu.add)
                            k -= pr
                        if first:
                            nc.vector.tensor_copy(out=agg[:],
                                                  in_=tmp[:, 0, :])
                            nc.vector.tensor_copy(out=den[:],
                                                  in_=sm[:, 0, :])
                            first = False
                        else:
                            nc.vector.tensor_tensor(out=agg[:], in0=agg[:],
                                                    in1=tmp[:, 0, :],
                                                    op=Alu.add)
                            nc.vector.tensor_tensor(out=den[:], in0=den[:],
                                                    in1=sm[:, 0, :],
                                                    op=Alu.add)
                    if sub < 5:
                        continue
                    # epilogue
                    den2 = spool.tile([128, heads], f32, tag="den2")
                    nc.vector.tensor_scalar(out=den2[:], in0=den[:],
                                            scalar1=1e-16, scalar2=None,
                                            op0=Alu.add)
                    rec = spool.tile([128, heads], f32, tag="rec")
                    nc.vector.reciprocal(out=rec[:], in_=den2[:])
                    o1 = wpool.tile([128, Fo], f32, tag="o1")
                    nc.vector.tensor_tensor(
                        out=o1[:].rearrange("p (h f) -> p h f", h=heads),
                        in0=agg[:].rearrange("p (h f) -> p h f", h=heads),
                        in1=rec[:].unsqueeze(-1)
                        .broadcast_to([128, heads, HID]),
                        op=Alu.mult)
                    o2 = wpool.tile([128, Fo], f32, tag="o2")
                    nc.vector.tensor_tensor(out=o2[:], in0=o1[:],
                                            in1=brep[:, :Fo], op=Alu.add)
                    if layer == 2:
                        nc.tensor.matmul(out=psum_sum[:],
                                         lhsT=maskc_s[:, b:b + 1],
                                         rhs=o2[:], start=(b == 0),
                                         stop=(b == NPOS - 1))
                    else:
                        o3 = wpool.tile([128, Fo], f32, tag="o3")
                        nc.scalar.activation(o3[:], o2[:], Act.Relu)
                        for k2 in range(2):
                            pt = ppool1.tile([128, 128], f32, tag="tp",
                                             space="PSUM")
                            nc.tensor.transpose(
                                pt[:], o3[:, k2 * 128:(k2 + 1) * 128],
                                identf_s[:])
                            nc.vector.tensor_copy(
                                out=hT[:, k2, b * BS:(b + 1) * BS],
                                in_=pt[:])
                if layer == 2:
                    osb = spool.tile([1, OUT_C], f32, tag="osb")
                    if sub >= 5:
                        nc.vector.tensor_copy(out=osb[:], in_=psum_sum[:])
                    else:
                        nc.vector.memset(osb[:], 0.0)
                    nc.sync.dma_start(out=out_d[:], in_=osb[:])

            upto = int(os.environ.get("GAT_UPTO", "99"))
            stage = 0
            for layer in range(3):
                if stage >= upto:
                    break
                transform(layer)
                stage += 1
                if stage >= upto:
                    break
                allgather(layer)
                stage += 1
                if stage >= upto:
                    break
                aggregate(layer)
                stage += 1

    nc.compile()
    return nc


_BUILT = None
_BUILT_KEY = None


def _get_built(pp):
    global _BUILT, _BUILT_KEY
    key = (tuple(pp["shapes"]), pp["totc"], pp["totm"])
    if _BUILT is None or _BUILT_KEY != key:
        _BUILT = build_kernel(pp)
        _BUILT_KEY = key
    return _BUILT


def kernel(**inputs) -> np.ndarray:
    from concourse.bass_utils import run_bass_kernel_spmd

    pp = preprocess(np.asarray(inputs["edge_index"]))
    in_maps = build_core_inputs(inputs, pp)
    nc = _get_built(pp)
    res = run_bass_kernel_spmd(nc, in_maps, core_ids=list(range(NCORES)))
    parts = np.stack([r["out_part"][0] for r in res.results])  # [8, 64]
    g = parts.sum(axis=0, keepdims=True) / N
    out = (g @ np.asarray(inputs["hw"], np.float32)
           + np.asarray(inputs["hb"], np.float32)).astype(np.float32)
    return out


# revision 32
# speedup vs baseline: 1.0231x; 1.0231x over previous
"""3-layer GAT on 8 trn2 NeuronCores.

Strategy (graph/data parallel per sharding hint):
  - Nodes are assigned to 8 cores x 49 blocks x 128 slots (degree-balanced
    LPT bin packing) -> permuted node order; "table row" = block*128 + slot.
  - Per layer: each core transforms its own node shard with
    rhs = [W | W@as | W@ad] (alpha terms folded into the matmul), writes a
    table shard [6272, F+2H(padded)], AllGather -> full table on every core.
  - Aggregation: per dst-block of 128 nodes, edges (dst-sorted) are packed
    into 128-edge tiles; a dma_gather fetches table rows for the tile's
    sources; a one-hot "scatter matrix" matmul accumulates both the
    s_e-weighted feature sum and the softmax denominator into PSUM.
    (Softmax max-shift is skipped: logits are O(1) so exp is safe, and the
    result is mathematically identical.)
  - int16 gather indices: table split into lo rows [0,32768) and hi rows
    [17408,50176); per-block edges are balanced between the (overlapping)
    windows so each side fits 9 tiles of 128.
  - Layer 2 output is column-summed per core (masked for pad slots); the
    final mean + linear head run on host.
"""

import os
import numpy as np

# ---------------- problem constants (must match reference) ----------------
N = 50000
E = 800000
IN_C = 128
HID = 64
HEADS = 4
OUT_C = 64
F1 = HEADS * HID  # 256

# ---------------- sharding geometry ----------------
NCORES = 8
NB = 49           # dst blocks per core
BS = 128          # dst slots per block
NPC = NB * BS     # 6272 nodes per core
RTOT = NCORES * NPC  # 50176 table rows
TL = 9            # tiles per kind (lo/hi)
KE = TL * 128     # 1152 edge slots per (block, kind)
LO_LIM = 32768    # lo window rows [0, LO_LIM)
HI_BASE = 17408   # hi window rows [HI_BASE, HI_BASE+32768)
NKCOLS = KE // 16  # 72 idx columns per (block, kind)

USE_BF16 = os.environ.get("GAT_BF16", "1") == "1"

if USE_BF16:
    import ml_dtypes
    TB_NP = ml_dtypes.bfloat16
    EL01 = 384     # table elems/row layer0/1 (256 h + 4 as + 4 ad + pad)
    EL2 = 128      # table elems/row layer2 (64 h + 1 as + 1 ad + pad)
else:
    TB_NP = np.float32
    EL01 = 320
    EL2 = 128


# ---------------- host preprocessing ----------------

def preprocess(edge_index):
    """Node->(core,block,slot) assignment and per-core edge tile arrays.

    Returns dict with:
      row:   [N] table row of each node
      xperm: [RTOT] node id occupying each table row (-1 for pad slots)
      idx16: [NCORES,128,NB*2*NKCOLS] int16 wrapped gather indices
      dstc:  [NCORES,128,NB*2*TL] f32 dst_local per edge slot (col layout, -1 pad)
      dstr:  [NCORES,128,KE] f32 dst_local (row layout; partition=block*2+kind)
      maskc: [NCORES,128,NB] f32 1.0 for real-node slots
    """
    import heapq

    src = np.concatenate([np.asarray(edge_index[0]), np.arange(N, dtype=np.int64)])
    dst = np.concatenate([np.asarray(edge_index[1]), np.arange(N, dtype=np.int64)])
    deg = np.bincount(dst, minlength=N)

    nblocks = NCORES * NB
    order = np.argsort(-deg, kind="stable")
    heap = [(0, b) for b in range(nblocks)]
    heapq.heapify(heap)
    slots_used = np.zeros(nblocks, np.int64)
    node_block = np.empty(N, np.int64)
    node_slot = np.empty(N, np.int64)
    for n in order:
        popped = []
        while True:
            load, b = heapq.heappop(heap)
            if slots_used[b] < BS:
                break
            popped.append((load, b))
        node_block[n] = b
        node_slot[n] = slots_used[b]
        slots_used[b] += 1
        heapq.heappush(heap, (load + int(deg[n]), b))
        # blocks that were full stay out of the heap

    row = node_block * BS + node_slot  # table row per node

    xperm = np.full(RTOT, -1, np.int64)
    xperm[row] = np.arange(N)

    erow = row[src]          # gather row per edge
    eblk = node_block[dst]   # destination block per edge
    eslot = node_slot[dst]   # dst_local per edge

    idx16 = np.zeros((NCORES, 128, NB * 2 * NKCOLS), np.int16)
    dstc = np.full((NCORES, 128, NB * 2 * TL), -1.0, np.float32)
    dstr = np.zeros((NCORES, 128, KE), np.float32)
    maskc = np.zeros((NCORES, 128, NB), np.float32)

    order_e = np.argsort(eblk, kind="stable")
    bounds = np.searchsorted(eblk[order_e], np.arange(nblocks + 1))

    for b in range(nblocks):
        c, bl = divmod(b, NB)
        es = order_e[bounds[b]:bounds[b + 1]]
        r_ = erow[es]
        dl = eslot[es]
        lo_f = r_ < HI_BASE
        hi_f = r_ >= LO_LIM
        flex = ~lo_f & ~hi_f
        n_lo = int(lo_f.sum())
        n_hi = int(hi_f.sum())
        n_fx = int(flex.sum())
        tot = n_lo + n_hi + n_fx
        assert tot <= 2 * KE, f"block {b} has {tot} edges > {2*KE}"
        # send flex edges to lo until lo reaches ceil(tot/2) (capped at KE)
        add_lo = min(n_fx, max(0, min(KE, (tot + 1) // 2) - n_lo))
        if n_hi + (n_fx - add_lo) > KE:
            add_lo = n_fx - (KE - n_hi)
        assert 0 <= add_lo <= n_fx
        fx_idx = np.nonzero(flex)[0]
        sel_lo = np.zeros(len(es), bool)
        sel_lo[lo_f] = True
        sel_lo[fx_idx[:add_lo]] = True
        sel_hi = ~sel_lo
        assert sel_lo.sum() <= KE and sel_hi.sum() <= KE, (
            b, sel_lo.sum(), sel_hi.sum())

        for kind, sel, base in ((0, sel_lo, 0), (1, sel_hi, HI_BASE)):
            rr = r_[sel]
            dd = dl[sel]
            o = np.argsort(rr, kind="stable")  # DMA locality
            rr = rr[o]
            dd = dd[o]
            k = len(rr)
            rel = np.zeros(KE, np.int64)
            rel[:k] = rr - base
            dloc = np.full(KE, -1.0, np.float32)
            dloc[:k] = dd.astype(np.float32)
            assert rel.min() >= 0 and rel.max() < 32768
            # wrapped idx: index i -> [i % 16, i // 16]
            w = rel.reshape(NKCOLS, 16).T.astype(np.int16)  # [16, NKCOLS]
            cbase = (bl * 2 + kind) * NKCOLS
            idx16[c, :, cbase:cbase + NKCOLS] = np.tile(w, (8, 1))
            # col layout: col bl*2*TL + kind*TL + t, partition p = edge t*128+p
            tcol = bl * 2 * TL + kind * TL
            dstc[c, :, tcol:tcol + TL] = dloc.reshape(TL, 128).T
            # row layout: partition bl*2+kind
            dstr[c, bl * 2 + kind, :] = dloc

        # mask of real slots
        used = slots_used[b]
        maskc[c, :used, bl] = 1.0

    return dict(row=row, xperm=xperm, idx16=idx16, dstc=dstc, dstr=dstr,
                maskc=maskc, deg=deg, node_block=node_block,
                node_slot=node_slot)


def host_weights(inputs):
    """Extended weight matrices with folded attention vectors."""
    def ext(W, a_s, a_d, heads):
        # Was[k, h] = sum_c W[k, h*HID+c] * a_s[h, c]
        Wh = W.reshape(W.shape[0], heads, HID)
        Was = np.einsum("khc,hc->kh", Wh, a_s)
        Wad = np.einsum("khc,hc->kh", Wh, a_d)
        return np.concatenate([W, Was, Wad], axis=1).astype(np.float32)

    W0e = ext(np.asarray(inputs["W0"], np.float32),
              np.asarray(inputs["a0s"], np.float32),
              np.asarray(inputs["a0d"], np.float32), HEADS)      # [128, 264]
    W1e = ext(np.asarray(inputs["W1"], np.float32),
              np.asarray(inputs["a1s"], np.float32),
              np.asarray(inputs["a1d"], np.float32), HEADS)      # [256, 264]
    W2e = ext(np.asarray(inputs["W2"], np.float32),
              np.asarray(inputs["a2s"], np.float32),
              np.asarray(inputs["a2d"], np.float32), 1)          # [256, 66]
    return W0e, W1e, W2e


def build_core_inputs(inputs, pp):
    """Per-core in_maps for run_bass_kernel_spmd."""
    x = np.asarray(inputs["x"], np.float32)
    W0e, W1e, W2e = host_weights(inputs)
    b0 = np.asarray(inputs["b0"], np.float32)
    b1 = np.asarray(inputs["b1"], np.float32)
    b2 = np.asarray(inputs["b2"], np.float32)

    iota_row = np.tile(np.arange(128, dtype=np.float32), (128, 1))
    iota_col = np.arange(128, dtype=np.float32).reshape(128, 1)
    ones1 = np.ones((1, 128), np.float32)
    ident = np.eye(128, dtype=np.float32)

    consts = dict(
        w0e=W0e,                                    # [128, 264]
        w1e=W1e.reshape(2, 128, F1 + 2 * HEADS),    # [2, 128, 264]
        w2e=W2e.reshape(2, 128, HID + 2),           # [2, 128, 66]
        b0r=np.tile(b0, (128, 1)).astype(np.float32),
        b1r=np.tile(b1, (128, 1)).astype(np.float32),
        b2r=np.tile(b2, (128, 1)).astype(np.float32),
        iota_row=iota_row, iota_col=iota_col, ones1=ones1, ident=ident,
    )

    in_maps = []
    for c in range(NCORES):
        # xTb[b] = x[nodes of (c,b)].T : [128 feats, 128 slots]
        xtb = np.zeros((NB, IN_C, BS), np.float32)
        rows = np.arange(c * NPC, (c + 1) * NPC)
        nodes = pp["xperm"][rows].reshape(NB, BS)
        for b in range(NB):
            nb = nodes[b]
            valid = nb >= 0
            if valid.any():
                xtb[b][:, valid] = x[nb[valid]].T
        m = dict(
            xtb=xtb,
            idx16=pp["idx16"][c],
            dstc=pp["dstc"][c],
            dstr=pp["dstr"][c],
            maskc=pp["maskc"][c],
            **consts,
        )
        in_maps.append(m)
    return in_maps


# ---------------- numpy emulation of the device data path ----------------

def _emulate_layer(tables_in, pp, We, brep, heads, F_out, relu, el):
    """tables_in: hT equivalent — full node-major feature mat [RTOT, F_in].
    Returns (out [RTOT, F_out] node-major post-activation, table [RTOT, el])."""
    Fi = We.shape[0]
    Fo = F_out * 1
    # transform (all rows; pad rows produce garbage but are never gathered)
    tb = tables_in @ We  # [RTOT, Fo + 2*heads]
    table = np.zeros((RTOT, el), TB_NP)
    table[:, :Fo + 2 * heads] = tb.astype(TB_NP)
    ad_all = tb[:, Fo + heads:Fo + 2 * heads]  # [RTOT, heads]

    out = np.zeros((RTOT, Fo), np.float32)
    for c in range(NCORES):
        for bl in range(NB):
            rbase = c * NPC + bl * BS
            agg = np.zeros((BS, Fo), np.float32)
            den = np.zeros((BS, heads), np.float32)
            for kind in range(2):
                base = 0 if kind == 0 else HI_BASE
                cbase = (bl * 2 + kind) * NKCOLS
                w = pp["idx16"][c][:16, cbase:cbase + NKCOLS]
                rel = w.T.reshape(-1).astype(np.int64)  # unwrap
                rows = rel + base
                g = np.asarray(table[rows], np.float32)  # [KE, el]
                dl = pp["dstr"][c][bl * 2 + kind].astype(np.int64)  # -1 pads
                valid = dl >= 0
                a_s = g[:, Fo:Fo + heads]
                a_d = np.where(valid[:, None], ad_all[rbase + dl], 0.0)
                z = a_s + a_d
                s = np.exp(np.maximum(z, 0.2 * z)).astype(np.float32)
                hsc = (g[:, :Fo].reshape(KE, heads, HID)
                       * s[:, :, None]).astype(TB_NP).astype(np.float32)
                hsc = hsc.reshape(KE, Fo)
                np.add.at(agg, dl[valid], hsc[valid])
                np.add.at(den, dl[valid], s[valid])
            o = agg.reshape(BS, heads, HID) / (den + 1e-16)[:, :, None]
            o = o.reshape(BS, Fo) + brep[0]
            if relu:
                o = np.maximum(o, 0.0)
            out[rbase:rbase + BS] = o
    return out


def emulate(inputs, pp=None):
    """Full numpy emulation; returns [1, OUT_C]."""
    if pp is None:
        pp = preprocess(np.asarray(inputs["edge_index"]))
    x = np.asarray(inputs["x"], np.float32)
    W0e, W1e, W2e = host_weights(inputs)
    h = np.zeros((RTOT, IN_C), np.float32)
    valid = pp["xperm"] >= 0
    h[valid] = x[pp["xperm"][valid]]

    b0r = np.tile(np.asarray(inputs["b0"], np.float32), (1, 1))
    b1r = np.tile(np.asarray(inputs["b1"], np.float32), (1, 1))
    b2r = np.tile(np.asarray(inputs["b2"], np.float32), (1, 1))

    h0 = _emulate_layer(h, pp, W0e, b0r, HEADS, F1, True, EL01)
    h1 = _emulate_layer(h0, pp, W1e, b1r, HEADS, F1, True, EL01)
    h2 = _emulate_layer(h1, pp, W2e, b2r, 1, HID, False, EL2)

    g = h2[valid].sum(axis=0, keepdims=True) / N
    return (g @ np.asarray(inputs["hw"], np.float32)
            + np.asarray(inputs["hb"], np.float32)).astype(np.float32)


# ---------------- device kernel ----------------

_BUILT = None


def build_kernel(upto=99):
    import concourse.bacc as bacc
    import concourse.bass as bass
    import concourse.mybir as mybir
    import concourse.tile as tile
    from concourse import library_config

    f32 = mybir.dt.float32
    tb_dt = mybir.dt.bfloat16 if USE_BF16 else mybir.dt.float32
    i16 = mybir.dt.int16
    Alu = mybir.AluOpType
    Act = mybir.ActivationFunctionType

    nc = bacc.Bacc("TRN2", target_bir_lowering=False, debug=False,
                   num_devices=NCORES, num_swdge_queues=4)

    # ---- I/O ----
    xtb_d = nc.dram_tensor("xtb", [NB, IN_C, BS], f32, kind="ExternalInput")
    idx16_d = nc.dram_tensor("idx16", [128, NB * 2 * NKCOLS], i16,
                             kind="ExternalInput")
    dstc_d = nc.dram_tensor("dstc", [128, NB * 2 * TL], f32,
                            kind="ExternalInput")
    dstr_d = nc.dram_tensor("dstr", [128, KE], f32, kind="ExternalInput")
    maskc_d = nc.dram_tensor("maskc", [128, NB], f32, kind="ExternalInput")
    w0e_d = nc.dram_tensor("w0e", [IN_C, F1 + 2 * HEADS], f32,
                           kind="ExternalInput")
    w1e_d = nc.dram_tensor("w1e", [2, 128, F1 + 2 * HEADS], f32,
                           kind="ExternalInput")
    w2e_d = nc.dram_tensor("w2e", [2, 128, HID + 2], f32,
                           kind="ExternalInput")
    b0r_d = nc.dram_tensor("b0r", [128, F1], f32, kind="ExternalInput")
    b1r_d = nc.dram_tensor("b1r", [128, F1], f32, kind="ExternalInput")
    b2r_d = nc.dram_tensor("b2r", [128, HID], f32, kind="ExternalInput")
    iota_row_d = nc.dram_tensor("iota_row", [128, 128], f32,
                                kind="ExternalInput")
    iota_col_d = nc.dram_tensor("iota_col", [128, 1], f32,
                                kind="ExternalInput")
    ones1_d = nc.dram_tensor("ones1", [1, 128], f32, kind="ExternalInput")
    ident_d = nc.dram_tensor("ident", [128, 128], f32, kind="ExternalInput")
    out_d = nc.dram_tensor("out_part", [1, OUT_C], f32, kind="ExternalOutput")
    debug = os.environ.get("GAT_DEBUG", "0") == "1"
    if debug:
        dmp_tb = nc.dram_tensor("dmp_tb", [NPC, EL01], f32,
                                kind="ExternalOutput")
        dmp_h = nc.dram_tensor("dmp_h", [NPC, F1], f32, kind="ExternalOutput")
        dmp_den = nc.dram_tensor("dmp_den", [NPC, HEADS], f32,
                                 kind="ExternalOutput")
        dmp_tmp = nc.dram_tensor("dmp_tmp", [128, TL, F1], f32,
                                 kind="ExternalOutput")
        dmp_agg = nc.dram_tensor("dmp_agg", [128, F1 + HEADS], f32,
                                 kind="ExternalOutput")
        dmp_g = nc.dram_tensor("dmp_g", [128, TL, EL01], f32,
                               kind="ExternalOutput")
        dmp_s = nc.dram_tensor("dmp_s", [128, TL * HEADS], f32,
                               kind="ExternalOutput")

    # internal DRAM
    shared = os.environ.get("GAT_SHARED", "1") == "1"
    kw = dict(addr_space="Shared") if shared else {}
    shard01 = nc.dram_tensor("shard01", [NPC, EL01], tb_dt)
    table01 = nc.dram_tensor("table01", [RTOT, EL01], tb_dt, **kw)
    shard2 = nc.dram_tensor("shard2", [NPC, EL2], tb_dt)
    table2 = nc.dram_tensor("table2", [RTOT, EL2], tb_dt, **kw)

    rg = [list(range(NCORES))]

    with tile.TileContext(nc) as tc:
        with (
            tc.tile_pool(name="const", bufs=1) as cpool,
            tc.tile_pool(name="big", bufs=1) as bigpool,
            tc.tile_pool(name="work", bufs=3) as wpool,
            tc.tile_pool(name="gather", bufs=3) as gpool,
            tc.tile_pool(name="small", bufs=4) as spool,
            tc.tile_pool(name="psum", bufs=2, space="PSUM") as ppool,
            tc.tile_pool(name="psum1", bufs=1, space="PSUM") as ppool1,
        ):
            # ---- load constants ----
            def load_const(tag, dram, shape, dtype=f32, view=None):
                t = cpool.tile(shape, dtype, tag=tag)
                nc.sync.dma_start(out=t[:], in_=view if view is not None
                                  else dram[:])
                return t

            w0e_s = load_const("w0e", w0e_d, [IN_C, F1 + 2 * HEADS])
            w1e_s = load_const("w1e", w1e_d, [128, 2, F1 + 2 * HEADS],
                               view=w1e_d[:].rearrange("c p j -> p c j"))
            w2e_s = load_const("w2e", w2e_d, [128, 2, HID + 2],
                               view=w2e_d[:].rearrange("c p j -> p c j"))
            b0r_s = load_const("b0r", b0r_d, [128, F1])
            b1r_s = load_const("b1r", b1r_d, [128, F1])
            b2r_s = load_const("b2r", b2r_d, [128, HID])
            iota_row_s = load_const("iota_row", iota_row_d, [128, 128])
            iota_col_s = load_const("iota_col", iota_col_d, [128, 1])
            ones1_s = load_const("ones1", ones1_d, [1, 128])
            ident_s = load_const("ident", ident_d, [128, 128])
            idx16_s = load_const("idx16", idx16_d,
                                 [128, NB * 2 * NKCOLS], i16)
            dstc_s = load_const("dstc", dstc_d, [128, NB * 2 * TL])
            maskc_s = load_const("maskc", maskc_d, [128, NB])

            nc.gpsimd.load_library(library_config.mlp)

            hT = bigpool.tile([128, 2, NPC], f32, tag="hT")

            def transform(layer):
                """Own-shard transform -> shard DRAM + ad_all SBUF."""
                heads = 1 if layer == 2 else HEADS
                Fo = HID if layer == 2 else F1
                ncols = Fo + 2 * heads
                el = EL2 if layer == 2 else EL01
                shard = shard2 if layer == 2 else shard01
                ad_all = spool.tile([128, NB * heads], f32, tag="ad_all")
                for b in range(NB):
                    ps = ppool.tile([128, 512], f32, tag="agg", space="PSUM")
                    if layer == 0:
                        xb = wpool.tile([IN_C, BS], f32, tag="xtb")
                        nc.sync.dma_start(out=xb[:], in_=xtb_d[b])
                        nc.tensor.matmul(out=ps[:, :ncols], lhsT=xb[:],
                                         rhs=w0e_s[:], start=True, stop=True)
                    else:
                        we = w1e_s if layer == 1 else w2e_s
                        for k2 in range(2):
                            nc.tensor.matmul(
                                out=ps[:, :ncols],
                                lhsT=hT[:, k2, b * BS:(b + 1) * BS],
                                rhs=we[:, k2, :],
                                start=(k2 == 0), stop=(k2 == 1))
                    tb = wpool.tile([128, el], tb_dt, tag="tbout")
                    nc.vector.tensor_copy(out=tb[:, :ncols],
                                          in_=ps[:, :ncols])
                    nc.vector.tensor_copy(
                        out=ad_all[:, b * heads:(b + 1) * heads],
                        in_=ps[:, Fo + heads:Fo + 2 * heads])
                    nc.sync.dma_start(out=shard[b * BS:(b + 1) * BS, :],
                                      in_=tb[:])
                    if debug and layer == 0:
                        nc.sync.dma_start(
                            out=dmp_tb[b * BS:(b + 1) * BS, :], in_=tb[:])
                return ad_all

            def allgather(layer):
                shard = shard2 if layer == 2 else shard01
                table = table2 if layer == 2 else table01
                nc.gpsimd.collective_compute(
                    "AllGather", mybir.AluOpType.bypass,
                    replica_groups=rg, ins=[shard[:].opt()],
                    outs=[table[:].opt()])

            def aggregate(layer, ad_all):
                sub = int(os.environ.get("GAT_AGG_SUB", "99"))
                heads = 1 if layer == 2 else HEADS
                Fo = HID if layer == 2 else F1
                el = EL2 if layer == 2 else EL01
                table = table2 if layer == 2 else table01
                brep = (b2r_s, b1r_s, b1r_s)[0] if False else (
                    b0r_s if layer == 0 else (b1r_s if layer == 1 else b2r_s))
                views = [table[0:LO_LIM, :], table[HI_BASE:HI_BASE + 32768, :]]
                if layer == 2:
                    psum_sum = ppool1.tile([1, OUT_C], f32, tag="sum",
                                          space="PSUM")
                for b in range(NB):
                    pagg = ppool.tile([128, Fo], f32, tag="agg",
                                      space="PSUM")
                    pden = ppool.tile([128, heads], f32, tag="den_ps",
                                      space="PSUM")
                    for kind in range(2):
                        bk = b * 2 + kind
                        g = gpool.tile([128, TL, el], tb_dt, tag="g")
                        nc.gpsimd.dma_gather(
                            g[:], views[kind],
                            idx16_s[:, bk * NKCOLS:(bk + 1) * NKCOLS],
                            KE, KE, el, single_packet=False,
                            queue_num=bk % 4)
                        if sub < 2:
                            continue
                        # one-hot M [128e, TL*128d]
                        M = wpool.tile([128, KE], tb_dt, tag="M")
                        tcol = b * 2 * TL + kind * TL
                        nc.vector.tensor_tensor(
                            out=M[:].rearrange("p (t d) -> p t d", t=TL),
                            in0=dstc_s[:, tcol:tcol + TL].unsqueeze(-1)
                                .broadcast_to([128, TL, 128]),
                            in1=iota_row_s[:].unsqueeze(1)
                                .broadcast_to([128, TL, 128]),
                            op=Alu.is_equal)
                        if sub < 3:
                            continue
                        # M_T [128d, TL*128e] via replicated-row outer product
                        MT = wpool.tile([128, KE], f32, tag="MT")
                        dr = spool.tile([1, KE], f32, tag="dr")
                        nc.sync.dma_start(out=dr[:], in_=dstr_d[bk:bk + 1, :])
                        for o, wdt in ((0, 512), (512, 512), (1024, 128)):
                            pr = ppool1.tile([128, 512], f32, tag="rep",
                                            space="PSUM")
                            nc.tensor.matmul(out=pr[:, :wdt],
                                             lhsT=ones1_s[:],
                                             rhs=dr[:, o:o + wdt],
                                             start=True, stop=True)
                            nc.vector.tensor_tensor(
                                out=MT[:, o:o + wdt], in0=pr[:, :wdt],
                                in1=iota_col_s[:]
                                    .broadcast_to([128, wdt]),
                                op=Alu.is_equal)
                        if sub < 4:
                            continue
                        # ad per edge via M_T @ ad_block
                        pad_ = ppool1.tile([128, TL * heads], f32, tag="adp",
                                          space="PSUM")
                        for t in range(TL):
                            nc.tensor.matmul(
                                out=pad_[:, t * heads:(t + 1) * heads],
                                lhsT=MT[:, t * 128:(t + 1) * 128],
                                rhs=ad_all[:, b * heads:(b + 1) * heads],
                                start=True, stop=True)
                        if sub < 5:
                            continue
                        # z = as + ad ; s = exp(max(z, 0.2 z))
                        z = spool.tile([128, TL * heads], f32, tag="z")
                        nc.vector.tensor_tensor(
                            out=z[:].rearrange("p (t h) -> p t h", t=TL),
                            in0=g[:, :, Fo:Fo + heads],
                            in1=pad_[:].rearrange("p (t h) -> p t h", t=TL),
                            op=Alu.add)
                        z2 = spool.tile([128, TL * heads], f32, tag="z2")
                        nc.vector.tensor_scalar(out=z2[:], in0=z[:],
                                                scalar1=0.2, scalar2=None,
                                                op0=Alu.mult)
                        zm = spool.tile([128, TL * heads], f32, tag="zm")
                        nc.vector.tensor_tensor(out=zm[:], in0=z[:],
                                                in1=z2[:], op=Alu.max)
                        s_t = spool.tile([128, TL * heads], tb_dt, tag="s")
                        nc.scalar.activation(s_t[:], zm[:], Act.Exp)
                        if sub < 6:
                            continue
                        # tmp = g[:, :, :Fo] * s (broadcast over HID),
                        # one 3D op per head (4D broadcast APs miscompute)
                        tmp = wpool.tile([128, TL, Fo], tb_dt, tag="tmp")
                        sv = s_t[:].rearrange("p (t h) -> p t h", t=TL)
                        for hh in range(heads):
                            nc.vector.tensor_tensor(
                                out=tmp[:, :, hh * HID:(hh + 1) * HID],
                                in0=g[:, :, hh * HID:(hh + 1) * HID],
                                in1=sv[:, :, hh:hh + 1]
                                    .broadcast_to([128, TL, HID]),
                                op=Alu.mult)
                        if debug and layer == 0 and b == 0 and kind == 0:
                            nc.sync.dma_start(out=dmp_tmp[:], in_=tmp[:])
                            nc.sync.dma_start(out=dmp_g[:], in_=g[:])
                            nc.sync.dma_start(out=dmp_s[:], in_=s_t[:])
                        if sub < 7:
                            continue
                        # accumulate
                        for t in range(TL):
                            first = (kind == 0 and t == 0)
                            last = (kind == 1 and t == TL - 1)
                            nc.tensor.matmul(
                                out=pagg[:],
                                lhsT=M[:, t * 128:(t + 1) * 128],
                                rhs=tmp[:, t, :],
                                start=first, stop=last)
                            nc.tensor.matmul(
                                out=pden[:],
                                lhsT=M[:, t * 128:(t + 1) * 128],
                                rhs=s_t[:, t * heads:(t + 1) * heads],
                                start=first, stop=last)
                    if sub < 8:
                        continue
                    # epilogue
                    if debug and layer == 0 and b == 0:
                        aggc = wpool.tile([128, F1 + HEADS], f32, tag="aggc")
                        nc.vector.tensor_copy(out=aggc[:, :F1], in_=pagg[:])
                        nc.vector.tensor_copy(out=aggc[:, F1:], in_=pden[:])
                        nc.sync.dma_start(out=dmp_agg[:], in_=aggc[:])
                    den = spool.tile([128, heads], f32, tag="den")
                    nc.vector.tensor_scalar(out=den[:],
                                            in0=pden[:],
                                            scalar1=1e-16, scalar2=None,
                                            op0=Alu.add)
                    rec = spool.tile([128, heads], f32, tag="rec")
                    nc.vector.reciprocal(out=rec[:], in_=den[:])
                    if debug and layer == 0:
                        nc.sync.dma_start(
                            out=dmp_den[b * BS:(b + 1) * BS, :], in_=den[:])
                    o1 = wpool.tile([128, Fo], f32, tag="o1")
                    nc.vector.tensor_tensor(
                        out=o1[:].rearrange("p (h f) -> p h f", h=heads),
                        in0=pagg[:].rearrange("p (h f) -> p h f",
                                              h=heads),
                        in1=rec[:].unsqueeze(-1)
                            .broadcast_to([128, heads, HID]),
                        op=Alu.mult)
                    o2 = wpool.tile([128, Fo], f32, tag="o2")
                    nc.vector.tensor_tensor(out=o2[:], in0=o1[:],
                                            in1=brep[:, :Fo], op=Alu.add)
                    if layer == 2:
                        nc.tensor.matmul(out=psum_sum[:],
                                         lhsT=maskc_s[:, b:b + 1],
                                         rhs=o2[:], start=(b == 0),
                                         stop=(b == NB - 1))
                    else:
                        o3 = wpool.tile([128, Fo], f32, tag="o3")
                        nc.scalar.activation(o3[:], o2[:], Act.Relu)
                        if debug and layer == 0:
                            nc.sync.dma_start(
                                out=dmp_h[b * BS:(b + 1) * BS, :], in_=o3[:])
                        for k2 in range(2):
                            pt = ppool1.tile([128, 128], f32, tag="tp",
                                            space="PSUM")
                            nc.tensor.transpose(
                                pt[:], o3[:, k2 * 128:(k2 + 1) * 128],
                                ident_s[:])
                            nc.vector.tensor_copy(
                                out=hT[:, k2, b * BS:(b + 1) * BS],
                                in_=pt[:])
                if layer == 2:
                    osb = spool.tile([1, OUT_C], f32, tag="osb")
                    nc.vector.tensor_copy(out=osb[:], in_=psum_sum[:])
                    nc.sync.dma_start(out=out_d[:], in_=osb[:])

            stage = 0
            for layer in range(3):
                if stage >= upto:
                    break
                ad_all = transform(layer)
                stage += 1
                if stage >= upto:
                    break
                allgather(layer)
                stage += 1
                if stage >= upto:
                    break
                aggregate(layer, ad_all)
                stage += 1

    nc.compile()
    return nc


def _get_built(pp=None):
    global _BUILT
    if _BUILT is None:
        _BUILT = build_kernel(upto=int(os.environ.get("GAT_UPTO", "99")))
    return _BUILT


def kernel(**inputs) -> np.ndarray:
    from concourse.bass_utils import run_bass_kernel_spmd

    pp = preprocess(np.asarray(inputs["edge_index"]))
    in_maps = build_core_inputs(inputs, pp)
    nc = _get_built()
    res = run_bass_kernel_spmd(nc, in_maps, core_ids=list(range(NCORES)))
    parts = np.stack([r["out_part"][0] for r in res.results])  # [8, 64]
    g = parts.sum(axis=0, keepdims=True) / N
    out = (g @ np.asarray(inputs["hw"], np.float32)
           + np.asarray(inputs["hb"], np.float32)).astype(np.float32)
    return out



# revision 34
# speedup vs baseline: 1.1098x; 1.0847x over previous
"""3-layer GAT on 8 trn2 NeuronCores.

Strategy (graph/data parallel per sharding hint):
  - Nodes are assigned to 8 cores x 49 blocks x 128 slots (degree-balanced
    LPT bin packing) -> permuted node order; "table row" = block*128 + slot.
  - Per layer: each core transforms its own node shard with
    rhs = [W | W@as | W@ad] (alpha terms folded into the matmul), writes a
    table shard [6272, F+2H(padded)], AllGather -> full table on every core.
  - Aggregation: per dst-block of 128 nodes, edges (dst-sorted) are packed
    into 128-edge tiles; a dma_gather fetches table rows for the tile's
    sources; a one-hot "scatter matrix" matmul accumulates both the
    s_e-weighted feature sum and the softmax denominator into PSUM.
    (Softmax max-shift is skipped: logits are O(1) so exp is safe, and the
    result is mathematically identical.)
  - int16 gather indices: table split into lo rows [0,32768) and hi rows
    [17408,50176); per-block edges are balanced between the (overlapping)
    windows so each side fits 9 tiles of 128.
  - Layer 2 output is column-summed per core (masked for pad slots); the
    final mean + linear head run on host.
"""

import os
import numpy as np

# ---------------- problem constants (must match reference) ----------------
N = 50000
E = 800000
IN_C = 128
HID = 64
HEADS = 4
OUT_C = 64
F1 = HEADS * HID  # 256

# ---------------- sharding geometry ----------------
NCORES = 8
NB = 49           # dst blocks per core
BS = 128          # dst slots per block
NPC = NB * BS     # 6272 nodes per core
RTOT = NCORES * NPC  # 50176 table rows
TL = 9            # tiles per kind (lo/hi)
KE = TL * 128     # 1152 edge slots per (block, kind)
LO_LIM = 32768    # lo window rows [0, LO_LIM)
HI_BASE = 17408   # hi window rows [HI_BASE, HI_BASE+32768)
NKCOLS = KE // 16  # 72 idx columns per (block, kind)

USE_BF16 = os.environ.get("GAT_BF16", "1") == "1"

if USE_BF16:
    import ml_dtypes
    TB_NP = ml_dtypes.bfloat16
    EL01 = 384     # table elems/row layer0/1 (256 h + 4 as + 4 ad + pad)
    EL2 = 128      # table elems/row layer2 (64 h + 1 as + 1 ad + pad)
else:
    TB_NP = np.float32
    EL01 = 320
    EL2 = 128


# ---------------- host preprocessing ----------------

def preprocess(edge_index):
    """Node->(core,block,slot) assignment and per-core edge tile arrays.

    Returns dict with:
      row:   [N] table row of each node
      xperm: [RTOT] node id occupying each table row (-1 for pad slots)
      idx16: [NCORES,128,NB*2*NKCOLS] int16 wrapped gather indices
      dstc:  [NCORES,128,NB*2*TL] f32 dst_local per edge slot (col layout, -1 pad)
      dstr:  [NCORES,128,KE] f32 dst_local (row layout; partition=block*2+kind)
      maskc: [NCORES,128,NB] f32 1.0 for real-node slots
    """
    import heapq

    src = np.concatenate([np.asarray(edge_index[0]), np.arange(N, dtype=np.int64)])
    dst = np.concatenate([np.asarray(edge_index[1]), np.arange(N, dtype=np.int64)])
    deg = np.bincount(dst, minlength=N)

    nblocks = NCORES * NB
    order = np.argsort(-deg, kind="stable")
    heap = [(0, b) for b in range(nblocks)]
    heapq.heapify(heap)
    slots_used = np.zeros(nblocks, np.int64)
    node_block = np.empty(N, np.int64)
    node_slot = np.empty(N, np.int64)
    for n in order:
        popped = []
        while True:
            load, b = heapq.heappop(heap)
            if slots_used[b] < BS:
                break
            popped.append((load, b))
        node_block[n] = b
        node_slot[n] = slots_used[b]
        slots_used[b] += 1
        heapq.heappush(heap, (load + int(deg[n]), b))
        # blocks that were full stay out of the heap

    row = node_block * BS + node_slot  # table row per node

    xperm = np.full(RTOT, -1, np.int64)
    xperm[row] = np.arange(N)

    erow = row[src]          # gather row per edge
    eblk = node_block[dst]   # destination block per edge
    eslot = node_slot[dst]   # dst_local per edge

    idx16 = np.zeros((NCORES, 128, NB * 2 * NKCOLS), np.int16)
    dstc = np.full((NCORES, 128, NB * 2 * TL), -1.0, np.float32)
    dstr = np.zeros((NCORES, 128, KE), np.float32)  # cast at build_core_inputs
    maskc = np.zeros((NCORES, 128, NB), np.float32)

    order_e = np.argsort(eblk, kind="stable")
    bounds = np.searchsorted(eblk[order_e], np.arange(nblocks + 1))

    for b in range(nblocks):
        c, bl = divmod(b, NB)
        es = order_e[bounds[b]:bounds[b + 1]]
        r_ = erow[es]
        dl = eslot[es]
        lo_f = r_ < HI_BASE
        hi_f = r_ >= LO_LIM
        flex = ~lo_f & ~hi_f
        n_lo = int(lo_f.sum())
        n_hi = int(hi_f.sum())
        n_fx = int(flex.sum())
        tot = n_lo + n_hi + n_fx
        assert tot <= 2 * KE, f"block {b} has {tot} edges > {2*KE}"
        # send flex edges to lo until lo reaches ceil(tot/2) (capped at KE)
        add_lo = min(n_fx, max(0, min(KE, (tot + 1) // 2) - n_lo))
        if n_hi + (n_fx - add_lo) > KE:
            add_lo = n_fx - (KE - n_hi)
        assert 0 <= add_lo <= n_fx
        fx_idx = np.nonzero(flex)[0]
        sel_lo = np.zeros(len(es), bool)
        sel_lo[lo_f] = True
        sel_lo[fx_idx[:add_lo]] = True
        sel_hi = ~sel_lo
        assert sel_lo.sum() <= KE and sel_hi.sum() <= KE, (
            b, sel_lo.sum(), sel_hi.sum())

        for kind, sel, base in ((0, sel_lo, 0), (1, sel_hi, HI_BASE)):
            rr = r_[sel]
            dd = dl[sel]
            o = np.argsort(rr, kind="stable")  # DMA locality
            rr = rr[o]
            dd = dd[o]
            k = len(rr)
            rel = np.zeros(KE, np.int64)
            rel[:k] = rr - base
            dloc = np.full(KE, -1.0, np.float32)
            dloc[:k] = dd.astype(np.float32)
            assert rel.min() >= 0 and rel.max() < 32768
            # wrapped idx: index i -> [i % 16, i // 16]
            w = rel.reshape(NKCOLS, 16).T.astype(np.int16)  # [16, NKCOLS]
            cbase = (bl * 2 + kind) * NKCOLS
            idx16[c, :, cbase:cbase + NKCOLS] = np.tile(w, (8, 1))
            # col layout: col bl*2*TL + kind*TL + t, partition p = edge t*128+p
            tcol = bl * 2 * TL + kind * TL
            dstc[c, :, tcol:tcol + TL] = dloc.reshape(TL, 128).T
            # row layout: partition bl*2+kind
            dstr[c, bl * 2 + kind, :] = dloc

        # mask of real slots
        used = slots_used[b]
        maskc[c, :used, bl] = 1.0

    return dict(row=row, xperm=xperm, idx16=idx16, dstc=dstc, dstr=dstr,
                maskc=maskc, deg=deg, node_block=node_block,
                node_slot=node_slot)


def host_weights(inputs):
    """Extended weight matrices with folded attention vectors."""
    def ext(W, a_s, a_d, heads):
        # Was[k, h] = sum_c W[k, h*HID+c] * a_s[h, c]
        Wh = W.reshape(W.shape[0], heads, HID)
        Was = np.einsum("khc,hc->kh", Wh, a_s)
        Wad = np.einsum("khc,hc->kh", Wh, a_d)
        return np.concatenate([W, Was, Wad], axis=1).astype(np.float32)

    W0e = ext(np.asarray(inputs["W0"], np.float32),
              np.asarray(inputs["a0s"], np.float32),
              np.asarray(inputs["a0d"], np.float32), HEADS)      # [128, 264]
    W1e = ext(np.asarray(inputs["W1"], np.float32),
              np.asarray(inputs["a1s"], np.float32),
              np.asarray(inputs["a1d"], np.float32), HEADS)      # [256, 264]
    W2e = ext(np.asarray(inputs["W2"], np.float32),
              np.asarray(inputs["a2s"], np.float32),
              np.asarray(inputs["a2d"], np.float32), 1)          # [256, 66]
    return W0e, W1e, W2e


def build_core_inputs(inputs, pp):
    """Per-core in_maps for run_bass_kernel_spmd."""
    x = np.asarray(inputs["x"], np.float32)
    W0e, W1e, W2e = host_weights(inputs)
    b0 = np.asarray(inputs["b0"], np.float32)
    b1 = np.asarray(inputs["b1"], np.float32)
    b2 = np.asarray(inputs["b2"], np.float32)

    iota_row = np.tile(np.arange(128, dtype=np.float32), (128, 1))
    iota_col = np.arange(128, dtype=np.float32).reshape(128, 1)
    ones1 = np.ones((1, 128), TB_NP)
    ident = np.eye(128, dtype=np.float32)

    consts = dict(
        w0e=W0e,                                    # [128, 264]
        w1e=W1e.reshape(2, 128, F1 + 2 * HEADS),    # [2, 128, 264]
        w2e=W2e.reshape(2, 128, HID + 2),           # [2, 128, 66]
        b0r=np.tile(b0, (128, 1)).astype(np.float32),
        b1r=np.tile(b1, (128, 1)).astype(np.float32),
        b2r=np.tile(b2, (128, 1)).astype(np.float32),
        iota_row=iota_row, iota_col=iota_col, ones1=ones1, ident=ident,
    )

    in_maps = []
    for c in range(NCORES):
        # xTb[b] = x[nodes of (c,b)].T : [128 feats, 128 slots]
        xtb = np.zeros((NB, IN_C, BS), np.float32)
        rows = np.arange(c * NPC, (c + 1) * NPC)
        nodes = pp["xperm"][rows].reshape(NB, BS)
        for b in range(NB):
            nb = nodes[b]
            valid = nb >= 0
            if valid.any():
                xtb[b][:, valid] = x[nb[valid]].T
        m = dict(
            xtb=xtb,
            idx16=pp["idx16"][c],
            dstc=pp["dstc"][c],
            dstr=pp["dstr"][c].astype(TB_NP),
            maskc=pp["maskc"][c],
            **consts,
        )
        in_maps.append(m)
    return in_maps


# ---------------- numpy emulation of the device data path ----------------

def _emulate_layer(tables_in, pp, We, brep, heads, F_out, relu, el):
    """tables_in: hT equivalent — full node-major feature mat [RTOT, F_in].
    Returns (out [RTOT, F_out] node-major post-activation, table [RTOT, el])."""
    Fi = We.shape[0]
    Fo = F_out * 1
    # transform (all rows; pad rows produce garbage but are never gathered)
    tb = tables_in @ We  # [RTOT, Fo + 2*heads]
    table = np.zeros((RTOT, el), TB_NP)
    table[:, :Fo + 2 * heads] = tb.astype(TB_NP)
    ad_all = tb[:, Fo + heads:Fo + 2 * heads]  # [RTOT, heads]

    out = np.zeros((RTOT, Fo), np.float32)
    for c in range(NCORES):
        for bl in range(NB):
            rbase = c * NPC + bl * BS
            agg = np.zeros((BS, Fo), np.float32)
            den = np.zeros((BS, heads), np.float32)
            for kind in range(2):
                base = 0 if kind == 0 else HI_BASE
                cbase = (bl * 2 + kind) * NKCOLS
                w = pp["idx16"][c][:16, cbase:cbase + NKCOLS]
                rel = w.T.reshape(-1).astype(np.int64)  # unwrap
                rows = rel + base
                g = np.asarray(table[rows], np.float32)  # [KE, el]
                dl = pp["dstr"][c][bl * 2 + kind].astype(np.int64)  # -1 pads
                valid = dl >= 0
                a_s = g[:, Fo:Fo + heads]
                a_d = np.where(valid[:, None], ad_all[rbase + dl], 0.0)
                z = a_s + a_d
                s = np.exp(np.maximum(z, 0.2 * z)).astype(np.float32)
                hsc = (g[:, :Fo].reshape(KE, heads, HID)
                       * s[:, :, None]).astype(TB_NP).astype(np.float32)
                hsc = hsc.reshape(KE, Fo)
                np.add.at(agg, dl[valid], hsc[valid])
                np.add.at(den, dl[valid], s[valid])
            o = agg.reshape(BS, heads, HID) / (den + 1e-16)[:, :, None]
            o = o.reshape(BS, Fo) + brep[0]
            if relu:
                o = np.maximum(o, 0.0)
            out[rbase:rbase + BS] = o
    return out


def emulate(inputs, pp=None):
    """Full numpy emulation; returns [1, OUT_C]."""
    if pp is None:
        pp = preprocess(np.asarray(inputs["edge_index"]))
    x = np.asarray(inputs["x"], np.float32)
    W0e, W1e, W2e = host_weights(inputs)
    h = np.zeros((RTOT, IN_C), np.float32)
    valid = pp["xperm"] >= 0
    h[valid] = x[pp["xperm"][valid]]

    b0r = np.tile(np.asarray(inputs["b0"], np.float32), (1, 1))
    b1r = np.tile(np.asarray(inputs["b1"], np.float32), (1, 1))
    b2r = np.tile(np.asarray(inputs["b2"], np.float32), (1, 1))

    h0 = _emulate_layer(h, pp, W0e, b0r, HEADS, F1, True, EL01)
    h1 = _emulate_layer(h0, pp, W1e, b1r, HEADS, F1, True, EL01)
    h2 = _emulate_layer(h1, pp, W2e, b2r, 1, HID, False, EL2)

    g = h2[valid].sum(axis=0, keepdims=True) / N
    return (g @ np.asarray(inputs["hw"], np.float32)
            + np.asarray(inputs["hb"], np.float32)).astype(np.float32)


# ---------------- device kernel ----------------

_BUILT = None


def build_kernel(upto=99):
    import concourse.bacc as bacc
    import concourse.bass as bass
    import concourse.mybir as mybir
    import concourse.tile as tile
    from concourse import library_config

    f32 = mybir.dt.float32
    tb_dt = mybir.dt.bfloat16 if USE_BF16 else mybir.dt.float32
    i16 = mybir.dt.int16
    Alu = mybir.AluOpType
    Act = mybir.ActivationFunctionType

    nc = bacc.Bacc("TRN2", target_bir_lowering=False, debug=False,
                   num_devices=NCORES, num_swdge_queues=4)

    # ---- I/O ----
    xtb_d = nc.dram_tensor("xtb", [NB, IN_C, BS], f32, kind="ExternalInput")
    idx16_d = nc.dram_tensor("idx16", [128, NB * 2 * NKCOLS], i16,
                             kind="ExternalInput")
    dstc_d = nc.dram_tensor("dstc", [128, NB * 2 * TL], f32,
                            kind="ExternalInput")
    dstr_d = nc.dram_tensor("dstr", [128, KE], tb_dt, kind="ExternalInput")
    maskc_d = nc.dram_tensor("maskc", [128, NB], f32, kind="ExternalInput")
    w0e_d = nc.dram_tensor("w0e", [IN_C, F1 + 2 * HEADS], f32,
                           kind="ExternalInput")
    w1e_d = nc.dram_tensor("w1e", [2, 128, F1 + 2 * HEADS], f32,
                           kind="ExternalInput")
    w2e_d = nc.dram_tensor("w2e", [2, 128, HID + 2], f32,
                           kind="ExternalInput")
    b0r_d = nc.dram_tensor("b0r", [128, F1], f32, kind="ExternalInput")
    b1r_d = nc.dram_tensor("b1r", [128, F1], f32, kind="ExternalInput")
    b2r_d = nc.dram_tensor("b2r", [128, HID], f32, kind="ExternalInput")
    iota_row_d = nc.dram_tensor("iota_row", [128, 128], f32,
                                kind="ExternalInput")
    iota_col_d = nc.dram_tensor("iota_col", [128, 1], f32,
                                kind="ExternalInput")
    ones1_d = nc.dram_tensor("ones1", [1, 128], tb_dt, kind="ExternalInput")
    ident_d = nc.dram_tensor("ident", [128, 128], f32, kind="ExternalInput")
    out_d = nc.dram_tensor("out_part", [1, OUT_C], f32, kind="ExternalOutput")
    debug = os.environ.get("GAT_DEBUG", "0") == "1"
    if debug:
        dmp_tb = nc.dram_tensor("dmp_tb", [NPC, EL01], f32,
                                kind="ExternalOutput")
        dmp_h = nc.dram_tensor("dmp_h", [NPC, F1], f32, kind="ExternalOutput")
        dmp_den = nc.dram_tensor("dmp_den", [NPC, HEADS], f32,
                                 kind="ExternalOutput")
        dmp_tmp = nc.dram_tensor("dmp_tmp", [128, TL, F1], f32,
                                 kind="ExternalOutput")
        dmp_agg = nc.dram_tensor("dmp_agg", [128, F1 + HEADS], f32,
                                 kind="ExternalOutput")
        dmp_g = nc.dram_tensor("dmp_g", [128, TL, EL01], f32,
                               kind="ExternalOutput")
        dmp_s = nc.dram_tensor("dmp_s", [128, TL * HEADS], f32,
                               kind="ExternalOutput")

    # internal DRAM
    shared = os.environ.get("GAT_SHARED", "1") == "1"
    kw = dict(addr_space="Shared") if shared else {}
    shard01 = nc.dram_tensor("shard01", [NPC, EL01], tb_dt)
    table01 = nc.dram_tensor("table01", [RTOT, EL01], tb_dt, **kw)
    shard2 = nc.dram_tensor("shard2", [NPC, EL2], tb_dt)
    table2 = nc.dram_tensor("table2", [RTOT, EL2], tb_dt, **kw)

    rg = [list(range(NCORES))]

    with tile.TileContext(nc) as tc:
        with (
            tc.tile_pool(name="const", bufs=1) as cpool,
            tc.tile_pool(name="big", bufs=1) as bigpool,
            tc.tile_pool(name="work", bufs=3) as wpool,
            tc.tile_pool(name="gather", bufs=3) as gpool,
            tc.tile_pool(name="small", bufs=4) as spool,
            tc.tile_pool(name="psum", bufs=2, space="PSUM") as ppool,
            tc.tile_pool(name="psum1", bufs=1, space="PSUM") as ppool1,
        ):
            # ---- load constants ----
            def load_const(tag, dram, shape, dtype=f32, view=None):
                t = cpool.tile(shape, dtype, tag=tag)
                nc.sync.dma_start(out=t[:], in_=view if view is not None
                                  else dram[:])
                return t

            w0e_s = load_const("w0e", w0e_d, [IN_C, F1 + 2 * HEADS])
            w1e_s = load_const("w1e", w1e_d, [128, 2, F1 + 2 * HEADS],
                               view=w1e_d[:].rearrange("c p j -> p c j"))
            w2e_s = load_const("w2e", w2e_d, [128, 2, HID + 2],
                               view=w2e_d[:].rearrange("c p j -> p c j"))
            b0r_s = load_const("b0r", b0r_d, [128, F1])
            b1r_s = load_const("b1r", b1r_d, [128, F1])
            b2r_s = load_const("b2r", b2r_d, [128, HID])
            iota_row_s = load_const("iota_row", iota_row_d, [128, 128])
            iota_col_s = load_const("iota_col", iota_col_d, [128, 1])
            ones1_s = load_const("ones1", ones1_d, [1, 128], tb_dt)
            ident_s = load_const("ident", ident_d, [128, 128])
            idx16_s = load_const("idx16", idx16_d,
                                 [128, NB * 2 * NKCOLS], i16)
            dstc_s = load_const("dstc", dstc_d, [128, NB * 2 * TL])
            maskc_s = load_const("maskc", maskc_d, [128, NB])

            nc.gpsimd.load_library(library_config.mlp)

            hT = bigpool.tile([128, 2, NPC], f32, tag="hT")

            def transform(layer):
                """Own-shard transform -> shard DRAM + ad_all SBUF."""
                heads = 1 if layer == 2 else HEADS
                Fo = HID if layer == 2 else F1
                ncols = Fo + 2 * heads
                el = EL2 if layer == 2 else EL01
                shard = shard2 if layer == 2 else shard01
                ad_all = spool.tile([128, NB * heads], tb_dt, tag="ad_all")
                for b in range(NB):
                    ps = ppool.tile([128, 512], f32, tag="agg", space="PSUM")
                    if layer == 0:
                        xb = wpool.tile([IN_C, BS], f32, tag="xtb")
                        nc.sync.dma_start(out=xb[:], in_=xtb_d[b])
                        nc.tensor.matmul(out=ps[:, :ncols], lhsT=xb[:],
                                         rhs=w0e_s[:], start=True, stop=True)
                    else:
                        we = w1e_s if layer == 1 else w2e_s
                        for k2 in range(2):
                            nc.tensor.matmul(
                                out=ps[:, :ncols],
                                lhsT=hT[:, k2, b * BS:(b + 1) * BS],
                                rhs=we[:, k2, :],
                                start=(k2 == 0), stop=(k2 == 1))
                    tb = wpool.tile([128, el], tb_dt, tag="tbout")
                    nc.vector.tensor_copy(out=tb[:, :ncols],
                                          in_=ps[:, :ncols])
                    nc.vector.tensor_copy(
                        out=ad_all[:, b * heads:(b + 1) * heads],
                        in_=ps[:, Fo + heads:Fo + 2 * heads])
                    nc.sync.dma_start(out=shard[b * BS:(b + 1) * BS, :],
                                      in_=tb[:])
                    if debug and layer == 0:
                        nc.sync.dma_start(
                            out=dmp_tb[b * BS:(b + 1) * BS, :], in_=tb[:])
                return ad_all

            def allgather(layer):
                shard = shard2 if layer == 2 else shard01
                table = table2 if layer == 2 else table01
                nc.gpsimd.collective_compute(
                    "AllGather", mybir.AluOpType.bypass,
                    replica_groups=rg, ins=[shard[:].opt()],
                    outs=[table[:].opt()])

            def aggregate(layer, ad_all):
                sub = int(os.environ.get("GAT_AGG_SUB", "99"))
                heads = 1 if layer == 2 else HEADS
                Fo = HID if layer == 2 else F1
                el = EL2 if layer == 2 else EL01
                table = table2 if layer == 2 else table01
                brep = (b2r_s, b1r_s, b1r_s)[0] if False else (
                    b0r_s if layer == 0 else (b1r_s if layer == 1 else b2r_s))
                views = [table[0:LO_LIM, :], table[HI_BASE:HI_BASE + 32768, :]]
                if layer == 2:
                    psum_sum = ppool1.tile([1, OUT_C], f32, tag="sum",
                                          space="PSUM")
                for b in range(NB):
                    pagg = ppool.tile([128, Fo], f32, tag="agg",
                                      space="PSUM")
                    pden = ppool.tile([128, heads], f32, tag="den_ps",
                                      space="PSUM")
                    for kind in range(2):
                        bk = b * 2 + kind
                        g = gpool.tile([128, TL, el], tb_dt, tag="g")
                        nc.gpsimd.dma_gather(
                            g[:], views[kind],
                            idx16_s[:, bk * NKCOLS:(bk + 1) * NKCOLS],
                            KE, KE, el, single_packet=False,
                            queue_num=bk % 4)
                        if sub < 2:
                            continue
                        # one-hot M [128e, TL*128d]
                        M = wpool.tile([128, KE], tb_dt, tag="M")
                        tcol = b * 2 * TL + kind * TL
                        nc.vector.tensor_tensor(
                            out=M[:].rearrange("p (t d) -> p t d", t=TL),
                            in0=dstc_s[:, tcol:tcol + TL].unsqueeze(-1)
                                .broadcast_to([128, TL, 128]),
                            in1=iota_row_s[:].unsqueeze(1)
                                .broadcast_to([128, TL, 128]),
                            op=Alu.is_equal)
                        if sub < 3:
                            continue
                        # M_T [128d, TL*128e] via replicated-row outer product
                        MT = wpool.tile([128, KE], tb_dt, tag="MT")
                        dr = spool.tile([1, KE], tb_dt, tag="dr")
                        nc.sync.dma_start(out=dr[:], in_=dstr_d[bk:bk + 1, :])
                        for o, wdt in ((0, 512), (512, 512), (1024, 128)):
                            pr = ppool1.tile([128, 512], f32, tag="rep",
                                            space="PSUM")
                            nc.tensor.matmul(out=pr[:, :wdt],
                                             lhsT=ones1_s[:],
                                             rhs=dr[:, o:o + wdt],
                                             start=True, stop=True)
                            nc.vector.tensor_tensor(
                                out=MT[:, o:o + wdt], in0=pr[:, :wdt],
                                in1=iota_col_s[:]
                                    .broadcast_to([128, wdt]),
                                op=Alu.is_equal)
                        if sub < 4:
                            continue
                        # ad per edge via M_T @ ad_block
                        pad_ = ppool1.tile([128, TL * heads], f32, tag="adp",
                                          space="PSUM")
                        for t in range(TL):
                            nc.tensor.matmul(
                                out=pad_[:, t * heads:(t + 1) * heads],
                                lhsT=MT[:, t * 128:(t + 1) * 128],
                                rhs=ad_all[:, b * heads:(b + 1) * heads],
                                start=True, stop=True)
                        if sub < 5:
                            continue
                        # z = as + ad ; s = exp(max(z, 0.2 z))
                        z = spool.tile([128, TL * heads], f32, tag="z")
                        nc.vector.tensor_tensor(
                            out=z[:].rearrange("p (t h) -> p t h", t=TL),
                            in0=g[:, :, Fo:Fo + heads],
                            in1=pad_[:].rearrange("p (t h) -> p t h", t=TL),
                            op=Alu.add)
                        z2 = spool.tile([128, TL * heads], f32, tag="z2")
                        nc.vector.tensor_scalar(out=z2[:], in0=z[:],
                                                scalar1=0.2, scalar2=None,
                                                op0=Alu.mult)
                        zm = spool.tile([128, TL * heads], f32, tag="zm")
                        nc.vector.tensor_tensor(out=zm[:], in0=z[:],
                                                in1=z2[:], op=Alu.max)
                        s_t = spool.tile([128, TL * heads], tb_dt, tag="s")
                        nc.scalar.activation(s_t[:], zm[:], Act.Exp)
                        if sub < 6:
                            continue
                        # tmp = g[:, :, :Fo] * s (broadcast over HID),
                        # one 3D op per head (4D broadcast APs miscompute)
                        tmp = wpool.tile([128, TL, Fo], tb_dt, tag="tmp")
                        sv = s_t[:].rearrange("p (t h) -> p t h", t=TL)
                        for hh in range(heads):
                            nc.vector.tensor_tensor(
                                out=tmp[:, :, hh * HID:(hh + 1) * HID],
                                in0=g[:, :, hh * HID:(hh + 1) * HID],
                                in1=sv[:, :, hh:hh + 1]
                                    .broadcast_to([128, TL, HID]),
                                op=Alu.mult)
                        if debug and layer == 0 and b == 0 and kind == 0:
                            nc.sync.dma_start(out=dmp_tmp[:], in_=tmp[:])
                            nc.sync.dma_start(out=dmp_g[:], in_=g[:])
                            nc.sync.dma_start(out=dmp_s[:], in_=s_t[:])
                        if sub < 7:
                            continue
                        # accumulate
                        for t in range(TL):
                            first = (kind == 0 and t == 0)
                            last = (kind == 1 and t == TL - 1)
                            nc.tensor.matmul(
                                out=pagg[:],
                                lhsT=M[:, t * 128:(t + 1) * 128],
                                rhs=tmp[:, t, :],
                                start=first, stop=last)
                            nc.tensor.matmul(
                                out=pden[:],
                                lhsT=M[:, t * 128:(t + 1) * 128],
                                rhs=s_t[:, t * heads:(t + 1) * heads],
                                start=first, stop=last)
                    if sub < 8:
                        continue
                    # epilogue
                    if debug and layer == 0 and b == 0:
                        aggc = wpool.tile([128, F1 + HEADS], f32, tag="aggc")
                        nc.vector.tensor_copy(out=aggc[:, :F1], in_=pagg[:])
                        nc.vector.tensor_copy(out=aggc[:, F1:], in_=pden[:])
                        nc.sync.dma_start(out=dmp_agg[:], in_=aggc[:])
                    den = spool.tile([128, heads], f32, tag="den")
                    nc.vector.tensor_scalar(out=den[:],
                                            in0=pden[:],
                                            scalar1=1e-16, scalar2=None,
                                            op0=Alu.add)
                    rec = spool.tile([128, heads], f32, tag="rec")
                    nc.vector.reciprocal(out=rec[:], in_=den[:])
                    if debug and layer == 0:
                        nc.sync.dma_start(
                            out=dmp_den[b * BS:(b + 1) * BS, :], in_=den[:])
                    o1 = wpool.tile([128, Fo], f32, tag="o1")
                    nc.vector.tensor_tensor(
                        out=o1[:].rearrange("p (h f) -> p h f", h=heads),
                        in0=pagg[:].rearrange("p (h f) -> p h f",
                                              h=heads),
                        in1=rec[:].unsqueeze(-1)
                            .broadcast_to([128, heads, HID]),
                        op=Alu.mult)
                    o2 = wpool.tile([128, Fo], f32, tag="o2")
                    nc.vector.tensor_tensor(out=o2[:], in0=o1[:],
                                            in1=brep[:, :Fo], op=Alu.add)
                    if layer == 2:
                        nc.tensor.matmul(out=psum_sum[:],
                                         lhsT=maskc_s[:, b:b + 1],
                                         rhs=o2[:], start=(b == 0),
                                         stop=(b == NB - 1))
                    else:
                        o3 = wpool.tile([128, Fo], f32, tag="o3")
                        nc.scalar.activation(o3[:], o2[:], Act.Relu)
                        if debug and layer == 0:
                            nc.sync.dma_start(
                                out=dmp_h[b * BS:(b + 1) * BS, :], in_=o3[:])
                        for k2 in range(2):
                            pt = ppool1.tile([128, 128], f32, tag="tp",
                                            space="PSUM")
                            nc.tensor.transpose(
                                pt[:], o3[:, k2 * 128:(k2 + 1) * 128],
                                ident_s[:])
                            nc.vector.tensor_copy(
                                out=hT[:, k2, b * BS:(b + 1) * BS],
                                in_=pt[:])
                if layer == 2:
                    osb = spool.tile([1, OUT_C], f32, tag="osb")
                    nc.vector.tensor_copy(out=osb[:], in_=psum_sum[:])
                    nc.sync.dma_start(out=out_d[:], in_=osb[:])

            stage = 0
            for layer in range(3):
                if stage >= upto:
                    break
                ad_all = transform(layer)
                stage += 1
                if stage >= upto:
                    break
                allgather(layer)
                stage += 1
                if stage >= upto:
                    break
                aggregate(layer, ad_all)
                stage += 1

    nc.compile()
    return nc


def _get_built(pp=None):
    global _BUILT
    if _BUILT is None:
        _BUILT = build_kernel(upto=int(os.environ.get("GAT_UPTO", "99")))
    return _BUILT


def kernel(**inputs) -> np.ndarray:
    from concourse.bass_utils import run_bass_kernel_spmd

    pp = preprocess(np.asarray(inputs["edge_index"]))
    in_maps = build_core_inputs(inputs, pp)
    nc = _get_built()
    res = run_bass_kernel_spmd(nc, in_maps, core_ids=list(range(NCORES)))
    parts = np.stack([r["out_part"][0] for r in res.results])  # [8, 64]
    g = parts.sum(axis=0, keepdims=True) / N
    out = (g @ np.asarray(inputs["hw"], np.float32)
           + np.asarray(inputs["hb"], np.float32)).astype(np.float32)
    return out



# revision 36
# speedup vs baseline: 1.6298x; 1.4686x over previous
"""3-layer GAT on 8 trn2 NeuronCores.

Strategy (graph/data parallel per sharding hint):
  - Nodes are assigned to 8 cores x 49 blocks x 128 slots (degree-balanced
    LPT bin packing) -> permuted node order; "table row" = block*128 + slot.
  - Per layer: each core transforms its own node shard with
    rhs = [W | W@as | W@ad] (alpha terms folded into the matmul), writes a
    table shard [6272, F+2H(padded)], AllGather -> full table on every core.
  - Aggregation: per dst-block of 128 nodes, edges (dst-sorted) are packed
    into 128-edge tiles; a dma_gather fetches table rows for the tile's
    sources; a one-hot "scatter matrix" matmul accumulates both the
    s_e-weighted feature sum and the softmax denominator into PSUM.
    (Softmax max-shift is skipped: logits are O(1) so exp is safe, and the
    result is mathematically identical.)
  - int16 gather indices: table split into lo rows [0,32768) and hi rows
    [17408,50176); per-block edges are balanced between the (overlapping)
    windows so each side fits 9 tiles of 128.
  - Layer 2 output is column-summed per core (masked for pad slots); the
    final mean + linear head run on host.
"""

import os
import numpy as np

# ---------------- problem constants (must match reference) ----------------
N = 50000
E = 800000
IN_C = 128
HID = 64
HEADS = 4
OUT_C = 64
F1 = HEADS * HID  # 256

# ---------------- sharding geometry ----------------
NCORES = 8
NB = 49           # dst blocks per core
BS = 128          # dst slots per block
NPC = NB * BS     # 6272 nodes per core
RTOT = NCORES * NPC  # 50176 table rows
TL = 9            # tiles per kind (lo/hi)
KE = TL * 128     # 1152 edge slots per (block, kind)
LO_LIM = 32768    # lo window rows [0, LO_LIM)
HI_BASE = 17408   # hi window rows [HI_BASE, HI_BASE+32768)
NKCOLS = KE // 16  # 72 idx columns per (block, kind)

USE_BF16 = os.environ.get("GAT_BF16", "1") == "1"

if USE_BF16:
    import ml_dtypes
    TB_NP = ml_dtypes.bfloat16
    EL01 = 384     # table elems/row layer0/1 (256 h + 4 as + 4 ad + pad)
    EL2 = 128      # table elems/row layer2 (64 h + 1 as + 1 ad + pad)
else:
    TB_NP = np.float32
    EL01 = 320
    EL2 = 128


# ---------------- host preprocessing ----------------

def preprocess(edge_index):
    """Node->(core,block,slot) assignment and per-core edge tile arrays.

    Returns dict with:
      row:   [N] table row of each node
      xperm: [RTOT] node id occupying each table row (-1 for pad slots)
      idx16: [NCORES,128,NB*2*NKCOLS] int16 wrapped gather indices
      dstc:  [NCORES,128,NB*2*TL] f32 dst_local per edge slot (col layout, -1 pad)
      dstr:  [NCORES,128,KE] f32 dst_local (row layout; partition=block*2+kind)
      maskc: [NCORES,128,NB] f32 1.0 for real-node slots
    """
    import heapq

    src = np.concatenate([np.asarray(edge_index[0]), np.arange(N, dtype=np.int64)])
    dst = np.concatenate([np.asarray(edge_index[1]), np.arange(N, dtype=np.int64)])
    deg = np.bincount(dst, minlength=N)

    nblocks = NCORES * NB
    order = np.argsort(-deg, kind="stable")
    heap = [(0, b) for b in range(nblocks)]
    heapq.heapify(heap)
    slots_used = np.zeros(nblocks, np.int64)
    node_block = np.empty(N, np.int64)
    node_slot = np.empty(N, np.int64)
    for n in order:
        popped = []
        while True:
            load, b = heapq.heappop(heap)
            if slots_used[b] < BS:
                break
            popped.append((load, b))
        node_block[n] = b
        node_slot[n] = slots_used[b]
        slots_used[b] += 1
        heapq.heappush(heap, (load + int(deg[n]), b))
        # blocks that were full stay out of the heap

    row = node_block * BS + node_slot  # table row per node

    xperm = np.full(RTOT, -1, np.int64)
    xperm[row] = np.arange(N)

    erow = row[src]          # gather row per edge
    eblk = node_block[dst]   # destination block per edge
    eslot = node_slot[dst]   # dst_local per edge

    idx16 = np.zeros((NCORES, 128, NB * 2 * NKCOLS), np.int16)
    dstc = np.full((NCORES, 128, NB * 2 * TL), -1.0, np.float32)
    dstr = np.zeros((NCORES, 128, KE), np.float32)  # cast at build_core_inputs
    maskc = np.zeros((NCORES, 128, NB), np.float32)

    order_e = np.argsort(eblk, kind="stable")
    bounds = np.searchsorted(eblk[order_e], np.arange(nblocks + 1))

    for b in range(nblocks):
        c, bl = divmod(b, NB)
        es = order_e[bounds[b]:bounds[b + 1]]
        r_ = erow[es]
        dl = eslot[es]
        lo_f = r_ < HI_BASE
        hi_f = r_ >= LO_LIM
        flex = ~lo_f & ~hi_f
        n_lo = int(lo_f.sum())
        n_hi = int(hi_f.sum())
        n_fx = int(flex.sum())
        tot = n_lo + n_hi + n_fx
        assert tot <= 2 * KE, f"block {b} has {tot} edges > {2*KE}"
        # send flex edges to lo until lo reaches ceil(tot/2) (capped at KE)
        add_lo = min(n_fx, max(0, min(KE, (tot + 1) // 2) - n_lo))
        if n_hi + (n_fx - add_lo) > KE:
            add_lo = n_fx - (KE - n_hi)
        assert 0 <= add_lo <= n_fx
        fx_idx = np.nonzero(flex)[0]
        sel_lo = np.zeros(len(es), bool)
        sel_lo[lo_f] = True
        sel_lo[fx_idx[:add_lo]] = True
        sel_hi = ~sel_lo
        assert sel_lo.sum() <= KE and sel_hi.sum() <= KE, (
            b, sel_lo.sum(), sel_hi.sum())

        for kind, sel, base in ((0, sel_lo, 0), (1, sel_hi, HI_BASE)):
            rr = r_[sel]
            dd = dl[sel]
            o = np.argsort(rr, kind="stable")  # DMA locality
            rr = rr[o]
            dd = dd[o]
            k = len(rr)
            rel = np.zeros(KE, np.int64)
            rel[:k] = rr - base
            dloc = np.full(KE, -1.0, np.float32)
            dloc[:k] = dd.astype(np.float32)
            assert rel.min() >= 0 and rel.max() < 32768
            # wrapped idx: index i -> [i % 16, i // 16]
            w = rel.reshape(NKCOLS, 16).T.astype(np.int16)  # [16, NKCOLS]
            cbase = (bl * 2 + kind) * NKCOLS
            idx16[c, :, cbase:cbase + NKCOLS] = np.tile(w, (8, 1))
            # col layout: col bl*2*TL + kind*TL + t, partition p = edge t*128+p
            tcol = bl * 2 * TL + kind * TL
            dstc[c, :, tcol:tcol + TL] = dloc.reshape(TL, 128).T
            # row layout: partition bl*2+kind
            dstr[c, bl * 2 + kind, :] = dloc

        # mask of real slots
        used = slots_used[b]
        maskc[c, :used, bl] = 1.0

    return dict(row=row, xperm=xperm, idx16=idx16, dstc=dstc, dstr=dstr,
                maskc=maskc, deg=deg, node_block=node_block,
                node_slot=node_slot)


def host_weights(inputs):
    """Extended weight matrices with folded attention vectors."""
    def ext(W, a_s, a_d, heads):
        # Was[k, h] = sum_c W[k, h*HID+c] * a_s[h, c]
        Wh = W.reshape(W.shape[0], heads, HID)
        Was = np.einsum("khc,hc->kh", Wh, a_s)
        Wad = np.einsum("khc,hc->kh", Wh, a_d)
        return np.concatenate([W, Was, Wad], axis=1).astype(np.float32)

    W0e = ext(np.asarray(inputs["W0"], np.float32),
              np.asarray(inputs["a0s"], np.float32),
              np.asarray(inputs["a0d"], np.float32), HEADS)      # [128, 264]
    W1e = ext(np.asarray(inputs["W1"], np.float32),
              np.asarray(inputs["a1s"], np.float32),
              np.asarray(inputs["a1d"], np.float32), HEADS)      # [256, 264]
    W2e = ext(np.asarray(inputs["W2"], np.float32),
              np.asarray(inputs["a2s"], np.float32),
              np.asarray(inputs["a2d"], np.float32), 1)          # [256, 66]
    return W0e, W1e, W2e


def build_core_inputs(inputs, pp):
    """Per-core in_maps for run_bass_kernel_spmd."""
    x = np.asarray(inputs["x"], np.float32)
    W0e, W1e, W2e = host_weights(inputs)
    b0 = np.asarray(inputs["b0"], np.float32)
    b1 = np.asarray(inputs["b1"], np.float32)
    b2 = np.asarray(inputs["b2"], np.float32)

    iota_row = np.tile(np.arange(128, dtype=np.float32), (128, 1))
    iota_col = np.arange(128, dtype=np.float32).reshape(128, 1)
    ones1 = np.ones((1, 128), TB_NP)
    ident = np.eye(128, dtype=np.float32)

    consts = dict(
        w0e=W0e,                                    # [128, 264]
        w1e=W1e.reshape(2, 128, F1 + 2 * HEADS),    # [2, 128, 264]
        w2e=W2e.reshape(2, 128, HID + 2),           # [2, 128, 66]
        b0r=np.tile(b0, (128, 1)).astype(np.float32),
        b1r=np.tile(b1, (128, 1)).astype(np.float32),
        b2r=np.tile(b2, (128, 1)).astype(np.float32),
        iota_row=iota_row, iota_col=iota_col, ones1=ones1, ident=ident,
    )

    in_maps = []
    for c in range(NCORES):
        # xTb[b] = x[nodes of (c,b)].T : [128 feats, 128 slots]
        xtb = np.zeros((NB, IN_C, BS), np.float32)
        rows = np.arange(c * NPC, (c + 1) * NPC)
        nodes = pp["xperm"][rows].reshape(NB, BS)
        for b in range(NB):
            nb = nodes[b]
            valid = nb >= 0
            if valid.any():
                xtb[b][:, valid] = x[nb[valid]].T
        m = dict(
            xtb=xtb,
            idx16=pp["idx16"][c],
            dstc=pp["dstc"][c],
            dstr=pp["dstr"][c].astype(TB_NP),
            maskc=pp["maskc"][c],
            **consts,
        )
        in_maps.append(m)
    return in_maps


# ---------------- numpy emulation of the device data path ----------------

def _emulate_layer(tables_in, pp, We, brep, heads, F_out, relu, el):
    """tables_in: hT equivalent — full node-major feature mat [RTOT, F_in].
    Returns (out [RTOT, F_out] node-major post-activation, table [RTOT, el])."""
    Fi = We.shape[0]
    Fo = F_out * 1
    # transform (all rows; pad rows produce garbage but are never gathered)
    tb = tables_in @ We  # [RTOT, Fo + 2*heads]
    table = np.zeros((RTOT, el), TB_NP)
    table[:, :Fo + 2 * heads] = tb.astype(TB_NP)
    ad_all = tb[:, Fo + heads:Fo + 2 * heads]  # [RTOT, heads]

    out = np.zeros((RTOT, Fo), np.float32)
    for c in range(NCORES):
        for bl in range(NB):
            rbase = c * NPC + bl * BS
            agg = np.zeros((BS, Fo), np.float32)
            den = np.zeros((BS, heads), np.float32)
            for kind in range(2):
                base = 0 if kind == 0 else HI_BASE
                cbase = (bl * 2 + kind) * NKCOLS
                w = pp["idx16"][c][:16, cbase:cbase + NKCOLS]
                rel = w.T.reshape(-1).astype(np.int64)  # unwrap
                rows = rel + base
                g = np.asarray(table[rows], np.float32)  # [KE, el]
                dl = pp["dstr"][c][bl * 2 + kind].astype(np.int64)  # -1 pads
                valid = dl >= 0
                a_s = g[:, Fo:Fo + heads]
                a_d = np.where(valid[:, None], ad_all[rbase + dl], 0.0)
                z = a_s + a_d
                s = np.exp(np.maximum(z, 0.2 * z)).astype(np.float32)
                hsc = (g[:, :Fo].reshape(KE, heads, HID)
                       * s[:, :, None]).astype(TB_NP).astype(np.float32)
                hsc = hsc.reshape(KE, Fo)
                np.add.at(agg, dl[valid], hsc[valid])
                np.add.at(den, dl[valid], s[valid])
            o = agg.reshape(BS, heads, HID) / (den + 1e-16)[:, :, None]
            o = o.reshape(BS, Fo) + brep[0]
            if relu:
                o = np.maximum(o, 0.0)
            out[rbase:rbase + BS] = o
    return out


def emulate(inputs, pp=None):
    """Full numpy emulation; returns [1, OUT_C]."""
    if pp is None:
        pp = preprocess(np.asarray(inputs["edge_index"]))
    x = np.asarray(inputs["x"], np.float32)
    W0e, W1e, W2e = host_weights(inputs)
    h = np.zeros((RTOT, IN_C), np.float32)
    valid = pp["xperm"] >= 0
    h[valid] = x[pp["xperm"][valid]]

    b0r = np.tile(np.asarray(inputs["b0"], np.float32), (1, 1))
    b1r = np.tile(np.asarray(inputs["b1"], np.float32), (1, 1))
    b2r = np.tile(np.asarray(inputs["b2"], np.float32), (1, 1))

    h0 = _emulate_layer(h, pp, W0e, b0r, HEADS, F1, True, EL01)
    h1 = _emulate_layer(h0, pp, W1e, b1r, HEADS, F1, True, EL01)
    h2 = _emulate_layer(h1, pp, W2e, b2r, 1, HID, False, EL2)

    g = h2[valid].sum(axis=0, keepdims=True) / N
    return (g @ np.asarray(inputs["hw"], np.float32)
            + np.asarray(inputs["hb"], np.float32)).astype(np.float32)


# ---------------- device kernel ----------------

_BUILT = None


def build_kernel(upto=99):
    import concourse.bacc as bacc
    import concourse.bass as bass
    import concourse.mybir as mybir
    import concourse.tile as tile
    from concourse import library_config

    f32 = mybir.dt.float32
    tb_dt = mybir.dt.bfloat16 if USE_BF16 else mybir.dt.float32
    i16 = mybir.dt.int16
    Alu = mybir.AluOpType
    Act = mybir.ActivationFunctionType

    nc = bacc.Bacc("TRN2", target_bir_lowering=False, debug=False,
                   num_devices=NCORES, num_swdge_queues=4)

    # ---- I/O ----
    xtb_d = nc.dram_tensor("xtb", [NB, IN_C, BS], f32, kind="ExternalInput")
    idx16_d = nc.dram_tensor("idx16", [128, NB * 2 * NKCOLS], i16,
                             kind="ExternalInput")
    dstc_d = nc.dram_tensor("dstc", [128, NB * 2 * TL], f32,
                            kind="ExternalInput")
    dstr_d = nc.dram_tensor("dstr", [128, KE], tb_dt, kind="ExternalInput")
    maskc_d = nc.dram_tensor("maskc", [128, NB], f32, kind="ExternalInput")
    w0e_d = nc.dram_tensor("w0e", [IN_C, F1 + 2 * HEADS], f32,
                           kind="ExternalInput")
    w1e_d = nc.dram_tensor("w1e", [2, 128, F1 + 2 * HEADS], f32,
                           kind="ExternalInput")
    w2e_d = nc.dram_tensor("w2e", [2, 128, HID + 2], f32,
                           kind="ExternalInput")
    b0r_d = nc.dram_tensor("b0r", [128, F1], f32, kind="ExternalInput")
    b1r_d = nc.dram_tensor("b1r", [128, F1], f32, kind="ExternalInput")
    b2r_d = nc.dram_tensor("b2r", [128, HID], f32, kind="ExternalInput")
    iota_row_d = nc.dram_tensor("iota_row", [128, 128], f32,
                                kind="ExternalInput")
    iota_col_d = nc.dram_tensor("iota_col", [128, 1], f32,
                                kind="ExternalInput")
    ones1_d = nc.dram_tensor("ones1", [1, 128], tb_dt, kind="ExternalInput")
    ident_d = nc.dram_tensor("ident", [128, 128], f32, kind="ExternalInput")
    out_d = nc.dram_tensor("out_part", [1, OUT_C], f32, kind="ExternalOutput")
    debug = os.environ.get("GAT_DEBUG", "0") == "1"
    if debug:
        dmp_tb = nc.dram_tensor("dmp_tb", [NPC, EL01], f32,
                                kind="ExternalOutput")
        dmp_h = nc.dram_tensor("dmp_h", [NPC, F1], f32, kind="ExternalOutput")
        dmp_den = nc.dram_tensor("dmp_den", [NPC, HEADS], f32,
                                 kind="ExternalOutput")
        dmp_tmp = nc.dram_tensor("dmp_tmp", [128, TL, F1], f32,
                                 kind="ExternalOutput")
        dmp_agg = nc.dram_tensor("dmp_agg", [128, F1 + HEADS], f32,
                                 kind="ExternalOutput")
        dmp_g = nc.dram_tensor("dmp_g", [128, TL, EL01], f32,
                               kind="ExternalOutput")
        dmp_s = nc.dram_tensor("dmp_s", [128, TL * HEADS], f32,
                               kind="ExternalOutput")

    # internal DRAM
    shared = os.environ.get("GAT_SHARED", "1") == "1"
    kw = dict(addr_space="Shared") if shared else {}
    shard01 = nc.dram_tensor("shard01", [NPC, EL01], tb_dt)
    table01 = nc.dram_tensor("table01", [RTOT, EL01], tb_dt, **kw)
    shard2 = nc.dram_tensor("shard2", [NPC, EL2], tb_dt)
    table2 = nc.dram_tensor("table2", [RTOT, EL2], tb_dt, **kw)

    rg = [list(range(NCORES))]

    with tile.TileContext(nc) as tc:
        with (
            tc.tile_pool(name="const", bufs=1) as cpool,
            tc.tile_pool(name="big", bufs=1) as bigpool,
            tc.tile_pool(name="work", bufs=3) as wpool,
            tc.tile_pool(name="gather", bufs=5) as gpool,
            tc.tile_pool(name="small", bufs=4) as spool,
            tc.tile_pool(name="psum", bufs=2, space="PSUM") as ppool,
            tc.tile_pool(name="psum1", bufs=1, space="PSUM") as ppool1,
        ):
            # ---- load constants ----
            def load_const(tag, dram, shape, dtype=f32, view=None):
                t = cpool.tile(shape, dtype, tag=tag)
                nc.sync.dma_start(out=t[:], in_=view if view is not None
                                  else dram[:])
                return t

            w0e_s = load_const("w0e", w0e_d, [IN_C, F1 + 2 * HEADS])
            w1e_s = load_const("w1e", w1e_d, [128, 2, F1 + 2 * HEADS],
                               view=w1e_d[:].rearrange("c p j -> p c j"))
            w2e_s = load_const("w2e", w2e_d, [128, 2, HID + 2],
                               view=w2e_d[:].rearrange("c p j -> p c j"))
            b0r_s = load_const("b0r", b0r_d, [128, F1])
            b1r_s = load_const("b1r", b1r_d, [128, F1])
            b2r_s = load_const("b2r", b2r_d, [128, HID])
            iota_row_s = load_const("iota_row", iota_row_d, [128, 128])
            iota_col_s = load_const("iota_col", iota_col_d, [128, 1])
            ones1_s = load_const("ones1", ones1_d, [1, 128], tb_dt)
            ident_s = load_const("ident", ident_d, [128, 128])
            idx16_s = load_const("idx16", idx16_d,
                                 [128, NB * 2 * NKCOLS], i16)
            dstc_s = load_const("dstc", dstc_d, [128, NB * 2 * TL])
            maskc_s = load_const("maskc", maskc_d, [128, NB])

            nc.gpsimd.load_library(library_config.mlp)

            hT = bigpool.tile([128, 2, NPC], f32, tag="hT")

            def transform(layer):
                """Own-shard transform -> shard DRAM + ad_all SBUF."""
                heads = 1 if layer == 2 else HEADS
                Fo = HID if layer == 2 else F1
                ncols = Fo + 2 * heads
                el = EL2 if layer == 2 else EL01
                shard = shard2 if layer == 2 else shard01
                ad_all = spool.tile([128, NB * heads], tb_dt, tag="ad_all")
                for b in range(NB):
                    ps = ppool.tile([128, 512], f32, tag="agg", space="PSUM")
                    if layer == 0:
                        xb = wpool.tile([IN_C, BS], f32, tag="xtb")
                        nc.sync.dma_start(out=xb[:], in_=xtb_d[b])
                        nc.tensor.matmul(out=ps[:, :ncols], lhsT=xb[:],
                                         rhs=w0e_s[:], start=True, stop=True)
                    else:
                        we = w1e_s if layer == 1 else w2e_s
                        for k2 in range(2):
                            nc.tensor.matmul(
                                out=ps[:, :ncols],
                                lhsT=hT[:, k2, b * BS:(b + 1) * BS],
                                rhs=we[:, k2, :],
                                start=(k2 == 0), stop=(k2 == 1))
                    tb = wpool.tile([128, el], tb_dt, tag="tbout")
                    nc.vector.tensor_copy(out=tb[:, :ncols],
                                          in_=ps[:, :ncols])
                    nc.vector.tensor_copy(
                        out=ad_all[:, b * heads:(b + 1) * heads],
                        in_=ps[:, Fo + heads:Fo + 2 * heads])
                    nc.sync.dma_start(out=shard[b * BS:(b + 1) * BS, :],
                                      in_=tb[:])
                    if debug and layer == 0:
                        nc.sync.dma_start(
                            out=dmp_tb[b * BS:(b + 1) * BS, :], in_=tb[:])
                return ad_all

            def allgather(layer):
                shard = shard2 if layer == 2 else shard01
                table = table2 if layer == 2 else table01
                nc.gpsimd.collective_compute(
                    "AllGather", mybir.AluOpType.bypass,
                    replica_groups=rg, ins=[shard[:].opt()],
                    outs=[table[:].opt()])

            def aggregate(layer, ad_all):
                sub = int(os.environ.get("GAT_AGG_SUB", "99"))
                heads = 1 if layer == 2 else HEADS
                Fo = HID if layer == 2 else F1
                el = EL2 if layer == 2 else EL01
                table = table2 if layer == 2 else table01
                brep = (b2r_s, b1r_s, b1r_s)[0] if False else (
                    b0r_s if layer == 0 else (b1r_s if layer == 1 else b2r_s))
                views = [table[0:LO_LIM, :], table[HI_BASE:HI_BASE + 32768, :]]
                if layer == 2:
                    psum_sum = ppool1.tile([1, OUT_C], f32, tag="sum",
                                          space="PSUM")
                for b in range(NB):
                    pagg = ppool.tile([128, Fo], f32, tag="agg",
                                      space="PSUM")
                    pden = ppool.tile([128, heads], f32, tag="den_ps",
                                      space="PSUM")
                    for kind in range(2):
                        bk = b * 2 + kind
                        g = gpool.tile([128, TL, el], tb_dt, tag="g")
                        nc.gpsimd.dma_gather(
                            g[:], views[kind],
                            idx16_s[:, bk * NKCOLS:(bk + 1) * NKCOLS],
                            KE, KE, el, single_packet=False,
                            queue_num=bk % 4)
                        if sub < 2:
                            continue
                        # one-hot M [128e, TL*128d]
                        M = wpool.tile([128, KE], tb_dt, tag="M")
                        tcol = b * 2 * TL + kind * TL
                        nc.vector.tensor_tensor(
                            out=M[:].rearrange("p (t d) -> p t d", t=TL),
                            in0=dstc_s[:, tcol:tcol + TL].unsqueeze(-1)
                                .broadcast_to([128, TL, 128]),
                            in1=iota_row_s[:].unsqueeze(1)
                                .broadcast_to([128, TL, 128]),
                            op=Alu.is_equal)
                        if sub < 3:
                            continue
                        # M_T [128d, TL*128e] via replicated-row outer product
                        MT = wpool.tile([128, KE], tb_dt, tag="MT")
                        dr = spool.tile([1, KE], tb_dt, tag="dr")
                        nc.sync.dma_start(out=dr[:], in_=dstr_d[bk:bk + 1, :])
                        for o, wdt in ((0, 512), (512, 512), (1024, 128)):
                            pr = ppool1.tile([128, 512], f32, tag="rep",
                                            space="PSUM")
                            nc.tensor.matmul(out=pr[:, :wdt],
                                             lhsT=ones1_s[:],
                                             rhs=dr[:, o:o + wdt],
                                             start=True, stop=True)
                            nc.vector.tensor_tensor(
                                out=MT[:, o:o + wdt], in0=pr[:, :wdt],
                                in1=iota_col_s[:]
                                    .broadcast_to([128, wdt]),
                                op=Alu.is_equal)
                        if sub < 4:
                            continue
                        # ad per edge via M_T @ ad_block
                        pad_ = ppool1.tile([128, TL * heads], f32, tag="adp",
                                          space="PSUM")
                        for t in range(TL):
                            nc.tensor.matmul(
                                out=pad_[:, t * heads:(t + 1) * heads],
                                lhsT=MT[:, t * 128:(t + 1) * 128],
                                rhs=ad_all[:, b * heads:(b + 1) * heads],
                                start=True, stop=True)
                        if sub < 5:
                            continue
                        # z = as + ad ; s = exp(max(z, 0.2 z))
                        z = spool.tile([128, TL * heads], f32, tag="z")
                        nc.vector.tensor_tensor(
                            out=z[:].rearrange("p (t h) -> p t h", t=TL),
                            in0=g[:, :, Fo:Fo + heads],
                            in1=pad_[:].rearrange("p (t h) -> p t h", t=TL),
                            op=Alu.add)
                        zm = spool.tile([128, TL * heads], f32, tag="zm")
                        nc.scalar.activation(zm[:], z[:], Act.Prelu,
                                             alpha=0.2)
                        s_t = spool.tile([128, TL * heads], tb_dt, tag="s")
                        nc.scalar.activation(s_t[:], zm[:], Act.Exp)
                        if sub < 6:
                            continue
                        # tmp = g[:, :, :Fo] * s (broadcast over HID),
                        # one 3D op per head (4D broadcast APs miscompute)
                        tmp = wpool.tile([128, TL, Fo], tb_dt, tag="tmp")
                        sv = s_t[:].rearrange("p (t h) -> p t h", t=TL)
                        for hh in range(heads):
                            nc.vector.tensor_tensor(
                                out=tmp[:, :, hh * HID:(hh + 1) * HID],
                                in0=g[:, :, hh * HID:(hh + 1) * HID],
                                in1=sv[:, :, hh:hh + 1]
                                    .broadcast_to([128, TL, HID]),
                                op=Alu.mult)
                        if debug and layer == 0 and b == 0 and kind == 0:
                            nc.sync.dma_start(out=dmp_tmp[:], in_=tmp[:])
                            nc.sync.dma_start(out=dmp_g[:], in_=g[:])
                            nc.sync.dma_start(out=dmp_s[:], in_=s_t[:])
                        if sub < 7:
                            continue
                        # accumulate
                        for t in range(TL):
                            first = (kind == 0 and t == 0)
                            last = (kind == 1 and t == TL - 1)
                            nc.tensor.matmul(
                                out=pagg[:],
                                lhsT=M[:, t * 128:(t + 1) * 128],
                                rhs=tmp[:, t, :],
                                start=first, stop=last)
                            nc.tensor.matmul(
                                out=pden[:],
                                lhsT=M[:, t * 128:(t + 1) * 128],
                                rhs=s_t[:, t * heads:(t + 1) * heads],
                                start=first, stop=last)
                    if sub < 8:
                        continue
                    # epilogue
                    if debug and layer == 0 and b == 0:
                        aggc = wpool.tile([128, F1 + HEADS], f32, tag="aggc")
                        nc.vector.tensor_copy(out=aggc[:, :F1], in_=pagg[:])
                        nc.vector.tensor_copy(out=aggc[:, F1:], in_=pden[:])
                        nc.sync.dma_start(out=dmp_agg[:], in_=aggc[:])
                    den = spool.tile([128, heads], f32, tag="den")
                    nc.vector.tensor_scalar(out=den[:],
                                            in0=pden[:],
                                            scalar1=1e-16, scalar2=None,
                                            op0=Alu.add)
                    rec = spool.tile([128, heads], f32, tag="rec")
                    nc.vector.reciprocal(out=rec[:], in_=den[:])
                    if debug and layer == 0:
                        nc.sync.dma_start(
                            out=dmp_den[b * BS:(b + 1) * BS, :], in_=den[:])
                    o1 = wpool.tile([128, Fo], f32, tag="o1")
                    nc.vector.tensor_tensor(
                        out=o1[:].rearrange("p (h f) -> p h f", h=heads),
                        in0=pagg[:].rearrange("p (h f) -> p h f",
                                              h=heads),
                        in1=rec[:].unsqueeze(-1)
                            .broadcast_to([128, heads, HID]),
                        op=Alu.mult)
                    o2 = wpool.tile([128, Fo], f32, tag="o2")
                    nc.vector.tensor_tensor(out=o2[:], in0=o1[:],
                                            in1=brep[:, :Fo], op=Alu.add)
                    if layer == 2:
                        nc.tensor.matmul(out=psum_sum[:],
                                         lhsT=maskc_s[:, b:b + 1],
                                         rhs=o2[:], start=(b == 0),
                                         stop=(b == NB - 1))
                    else:
                        o3 = wpool.tile([128, Fo], f32, tag="o3")
                        nc.scalar.activation(o3[:], o2[:], Act.Relu)
                        if debug and layer == 0:
                            nc.sync.dma_start(
                                out=dmp_h[b * BS:(b + 1) * BS, :], in_=o3[:])
                        for k2 in range(2):
                            pt = ppool1.tile([128, 128], f32, tag="tp",
                                            space="PSUM")
                            nc.tensor.transpose(
                                pt[:], o3[:, k2 * 128:(k2 + 1) * 128],
                                ident_s[:])
                            nc.vector.tensor_copy(
                                out=hT[:, k2, b * BS:(b + 1) * BS],
                                in_=pt[:])
                if layer == 2:
                    osb = spool.tile([1, OUT_C], f32, tag="osb")
                    nc.vector.tensor_copy(out=osb[:], in_=psum_sum[:])
                    nc.sync.dma_start(out=out_d[:], in_=osb[:])

            stage = 0
            for layer in range(3):
                if stage >= upto:
                    break
                ad_all = transform(layer)
                stage += 1
                if stage >= upto:
                    break
                allgather(layer)
                stage += 1
                if stage >= upto:
                    break
                aggregate(layer, ad_all)
                stage += 1

    nc.compile()
    return nc


def _get_built(pp=None):
    global _BUILT
    if _BUILT is None:
        _BUILT = build_kernel(upto=int(os.environ.get("GAT_UPTO", "99")))
    return _BUILT


def kernel(**inputs) -> np.ndarray:
    from concourse.bass_utils import run_bass_kernel_spmd

    pp = preprocess(np.asarray(inputs["edge_index"]))
    in_maps = build_core_inputs(inputs, pp)
    nc = _get_built()
    res = run_bass_kernel_spmd(nc, in_maps, core_ids=list(range(NCORES)))
    parts = np.stack([r["out_part"][0] for r in res.results])  # [8, 64]
    g = parts.sum(axis=0, keepdims=True) / N
    out = (g @ np.asarray(inputs["hw"], np.float32)
           + np.asarray(inputs["hb"], np.float32)).astype(np.float32)
    return out



# revision 39
# speedup vs baseline: 1.6519x; 1.0135x over previous
"""3-layer GAT on 8 trn2 NeuronCores.

Strategy (graph/data parallel per sharding hint):
  - Nodes are assigned to 8 cores x 49 blocks x 128 slots (degree-balanced
    LPT bin packing) -> permuted node order; "table row" = block*128 + slot.
  - Per layer: each core transforms its own node shard with
    rhs = [W | W@as | W@ad] (alpha terms folded into the matmul), writes a
    table shard [6272, F+2H(padded)], AllGather -> full table on every core.
  - Aggregation: per dst-block of 128 nodes, edges (dst-sorted) are packed
    into 128-edge tiles; a dma_gather fetches table rows for the tile's
    sources; a one-hot "scatter matrix" matmul accumulates both the
    s_e-weighted feature sum and the softmax denominator into PSUM.
    (Softmax max-shift is skipped: logits are O(1) so exp is safe, and the
    result is mathematically identical.)
  - int16 gather indices: table split into lo rows [0,32768) and hi rows
    [17408,50176); per-block edges are balanced between the (overlapping)
    windows so each side fits 9 tiles of 128.
  - Layer 2 output is column-summed per core (masked for pad slots); the
    final mean + linear head run on host.
"""

import os
import numpy as np

# ---------------- problem constants (must match reference) ----------------
N = 50000
E = 800000
IN_C = 128
HID = 64
HEADS = 4
OUT_C = 64
F1 = HEADS * HID  # 256

# ---------------- sharding geometry ----------------
NCORES = 8
NB = 49           # dst blocks per core
BS = 128          # dst slots per block
NPC = NB * BS     # 6272 nodes per core
RTOT = NCORES * NPC  # 50176 table rows
TL = 9            # tiles per kind (lo/hi)
KE = TL * 128     # 1152 edge slots per (block, kind)
LO_LIM = 32768    # lo window rows [0, LO_LIM)
HI_BASE = 17408   # hi window rows [HI_BASE, HI_BASE+32768)
NKCOLS = KE // 16  # 72 idx columns per (block, kind)

USE_BF16 = os.environ.get("GAT_BF16", "1") == "1"

if USE_BF16:
    import ml_dtypes
    TB_NP = ml_dtypes.bfloat16
    EL01 = 384     # table elems/row layer0/1 (256 h + 4 as + 4 ad + pad)
    EL2 = 128      # table elems/row layer2 (64 h + 1 as + 1 ad + pad)
else:
    TB_NP = np.float32
    EL01 = 320
    EL2 = 128


# ---------------- host preprocessing ----------------

def preprocess(edge_index):
    """Node->(core,block,slot) assignment and per-core edge tile arrays.

    Returns dict with:
      row:   [N] table row of each node
      xperm: [RTOT] node id occupying each table row (-1 for pad slots)
      idx16: [NCORES,128,NB*2*NKCOLS] int16 wrapped gather indices
      dstc:  [NCORES,128,NB*2*TL] f32 dst_local per edge slot (col layout, -1 pad)
      dstr:  [NCORES,128,KE] f32 dst_local (row layout; partition=block*2+kind)
      maskc: [NCORES,128,NB] f32 1.0 for real-node slots
    """
    import heapq

    src = np.concatenate([np.asarray(edge_index[0]), np.arange(N, dtype=np.int64)])
    dst = np.concatenate([np.asarray(edge_index[1]), np.arange(N, dtype=np.int64)])
    deg = np.bincount(dst, minlength=N)

    nblocks = NCORES * NB
    order = np.argsort(-deg, kind="stable")
    heap = [(0, b) for b in range(nblocks)]
    heapq.heapify(heap)
    slots_used = np.zeros(nblocks, np.int64)
    node_block = np.empty(N, np.int64)
    node_slot = np.empty(N, np.int64)
    for n in order:
        popped = []
        while True:
            load, b = heapq.heappop(heap)
            if slots_used[b] < BS:
                break
            popped.append((load, b))
        node_block[n] = b
        node_slot[n] = slots_used[b]
        slots_used[b] += 1
        heapq.heappush(heap, (load + int(deg[n]), b))
        # blocks that were full stay out of the heap

    row = node_block * BS + node_slot  # table row per node

    xperm = np.full(RTOT, -1, np.int64)
    xperm[row] = np.arange(N)

    erow = row[src]          # gather row per edge
    eblk = node_block[dst]   # destination block per edge
    eslot = node_slot[dst]   # dst_local per edge

    idx16 = np.zeros((NCORES, 128, NB * 2 * NKCOLS), np.int16)
    kreal = np.zeros(NB * 2, np.int64)  # max real idx count per (block,kind)
    dstc = np.full((NCORES, 128, NB * 2 * TL), -1.0, np.float32)
    dstr = np.zeros((NCORES, 128, KE), np.float32)  # cast at build_core_inputs
    maskc = np.zeros((NCORES, 128, NB), np.float32)

    order_e = np.argsort(eblk, kind="stable")
    bounds = np.searchsorted(eblk[order_e], np.arange(nblocks + 1))

    for b in range(nblocks):
        c, bl = divmod(b, NB)
        es = order_e[bounds[b]:bounds[b + 1]]
        r_ = erow[es]
        dl = eslot[es]
        lo_f = r_ < HI_BASE
        hi_f = r_ >= LO_LIM
        flex = ~lo_f & ~hi_f
        n_lo = int(lo_f.sum())
        n_hi = int(hi_f.sum())
        n_fx = int(flex.sum())
        tot = n_lo + n_hi + n_fx
        assert tot <= 2 * KE, f"block {b} has {tot} edges > {2*KE}"
        # send flex edges to lo until lo reaches ceil(tot/2) (capped at KE)
        add_lo = min(n_fx, max(0, min(KE, (tot + 1) // 2) - n_lo))
        if n_hi + (n_fx - add_lo) > KE:
            add_lo = n_fx - (KE - n_hi)
        assert 0 <= add_lo <= n_fx
        fx_idx = np.nonzero(flex)[0]
        sel_lo = np.zeros(len(es), bool)
        sel_lo[lo_f] = True
        sel_lo[fx_idx[:add_lo]] = True
        sel_hi = ~sel_lo
        assert sel_lo.sum() <= KE and sel_hi.sum() <= KE, (
            b, sel_lo.sum(), sel_hi.sum())

        for kind, sel, base in ((0, sel_lo, 0), (1, sel_hi, HI_BASE)):
            rr = r_[sel]
            dd = dl[sel]
            o = np.argsort(rr, kind="stable")  # DMA locality
            rr = rr[o]
            dd = dd[o]
            k = len(rr)
            rel = np.zeros(KE, np.int64)
            rel[:k] = rr - base
            dloc = np.full(KE, -1.0, np.float32)
            dloc[:k] = dd.astype(np.float32)
            assert rel.min() >= 0 and rel.max() < 32768
            # wrapped idx: index i -> [i % 16, i // 16]
            w = rel.reshape(NKCOLS, 16).T.astype(np.int16)  # [16, NKCOLS]
            cbase = (bl * 2 + kind) * NKCOLS
            idx16[c, :, cbase:cbase + NKCOLS] = np.tile(w, (8, 1))
            # col layout: col bl*2*TL + kind*TL + t, partition p = edge t*128+p
            tcol = bl * 2 * TL + kind * TL
            dstc[c, :, tcol:tcol + TL] = dloc.reshape(TL, 128).T
            # row layout: partition bl*2+kind
            dstr[c, bl * 2 + kind, :] = dloc
            kreal[bl * 2 + kind] = max(kreal[bl * 2 + kind], k)

        # mask of real slots
        used = slots_used[b]
        maskc[c, :used, bl] = 1.0

    return dict(row=row, xperm=xperm, idx16=idx16, dstc=dstc, dstr=dstr,
                maskc=maskc, deg=deg, node_block=node_block,
                node_slot=node_slot, kreal=kreal)


def host_weights(inputs):
    """Extended weight matrices with folded attention vectors."""
    def ext(W, a_s, a_d, heads):
        # Was[k, h] = sum_c W[k, h*HID+c] * a_s[h, c]
        Wh = W.reshape(W.shape[0], heads, HID)
        Was = np.einsum("khc,hc->kh", Wh, a_s)
        Wad = np.einsum("khc,hc->kh", Wh, a_d)
        return np.concatenate([W, Was, Wad], axis=1).astype(np.float32)

    W0e = ext(np.asarray(inputs["W0"], np.float32),
              np.asarray(inputs["a0s"], np.float32),
              np.asarray(inputs["a0d"], np.float32), HEADS)      # [128, 264]
    W1e = ext(np.asarray(inputs["W1"], np.float32),
              np.asarray(inputs["a1s"], np.float32),
              np.asarray(inputs["a1d"], np.float32), HEADS)      # [256, 264]
    W2e = ext(np.asarray(inputs["W2"], np.float32),
              np.asarray(inputs["a2s"], np.float32),
              np.asarray(inputs["a2d"], np.float32), 1)          # [256, 66]
    return W0e, W1e, W2e


def build_core_inputs(inputs, pp):
    """Per-core in_maps for run_bass_kernel_spmd."""
    x = np.asarray(inputs["x"], np.float32)
    W0e, W1e, W2e = host_weights(inputs)
    b0 = np.asarray(inputs["b0"], np.float32)
    b1 = np.asarray(inputs["b1"], np.float32)
    b2 = np.asarray(inputs["b2"], np.float32)

    iota_row = np.tile(np.arange(128, dtype=np.float32), (128, 1))
    iota_col = np.arange(128, dtype=np.float32).reshape(128, 1)
    ones1 = np.ones((1, 128), TB_NP)
    ident = np.eye(128, dtype=np.float32)

    consts = dict(
        w0e=W0e,                                    # [128, 264]
        w1e=W1e.reshape(2, 128, F1 + 2 * HEADS),    # [2, 128, 264]
        w2e=W2e.reshape(2, 128, HID + 2),           # [2, 128, 66]
        b0r=np.tile(b0, (128, 1)).astype(np.float32),
        b1r=np.tile(b1, (128, 1)).astype(np.float32),
        b2r=np.tile(b2, (128, 1)).astype(np.float32),
        iota_row=iota_row, iota_col=iota_col, ones1=ones1, ident=ident,
    )

    in_maps = []
    for c in range(NCORES):
        # xTb[b] = x[nodes of (c,b)].T : [128 feats, 128 slots]
        xtb = np.zeros((NB, IN_C, BS), np.float32)
        rows = np.arange(c * NPC, (c + 1) * NPC)
        nodes = pp["xperm"][rows].reshape(NB, BS)
        for b in range(NB):
            nb = nodes[b]
            valid = nb >= 0
            if valid.any():
                xtb[b][:, valid] = x[nb[valid]].T
        m = dict(
            xtb=xtb,
            idx16=pp["idx16"][c],
            dstc=pp["dstc"][c],
            dstr=pp["dstr"][c].astype(TB_NP),
            maskc=pp["maskc"][c],
            **consts,
        )
        in_maps.append(m)
    return in_maps


# ---------------- numpy emulation of the device data path ----------------

def _emulate_layer(tables_in, pp, We, brep, heads, F_out, relu, el):
    """tables_in: hT equivalent — full node-major feature mat [RTOT, F_in].
    Returns (out [RTOT, F_out] node-major post-activation, table [RTOT, el])."""
    Fi = We.shape[0]
    Fo = F_out * 1
    # transform (all rows; pad rows produce garbage but are never gathered)
    tb = tables_in @ We  # [RTOT, Fo + 2*heads]
    table = np.zeros((RTOT, el), TB_NP)
    table[:, :Fo + 2 * heads] = tb.astype(TB_NP)
    ad_all = tb[:, Fo + heads:Fo + 2 * heads]  # [RTOT, heads]

    out = np.zeros((RTOT, Fo), np.float32)
    for c in range(NCORES):
        for bl in range(NB):
            rbase = c * NPC + bl * BS
            agg = np.zeros((BS, Fo), np.float32)
            den = np.zeros((BS, heads), np.float32)
            for kind in range(2):
                base = 0 if kind == 0 else HI_BASE
                cbase = (bl * 2 + kind) * NKCOLS
                w = pp["idx16"][c][:16, cbase:cbase + NKCOLS]
                rel = w.T.reshape(-1).astype(np.int64)  # unwrap
                rows = rel + base
                g = np.asarray(table[rows], np.float32)  # [KE, el]
                dl = pp["dstr"][c][bl * 2 + kind].astype(np.int64)  # -1 pads
                valid = dl >= 0
                a_s = g[:, Fo:Fo + heads]
                a_d = np.where(valid[:, None], ad_all[rbase + dl], 0.0)
                z = a_s + a_d
                s = np.exp(np.maximum(z, 0.2 * z)).astype(np.float32)
                hsc = (g[:, :Fo].reshape(KE, heads, HID)
                       * s[:, :, None]).astype(TB_NP).astype(np.float32)
                hsc = hsc.reshape(KE, Fo)
                np.add.at(agg, dl[valid], hsc[valid])
                np.add.at(den, dl[valid], s[valid])
            o = agg.reshape(BS, heads, HID) / (den + 1e-16)[:, :, None]
            o = o.reshape(BS, Fo) + brep[0]
            if relu:
                o = np.maximum(o, 0.0)
            out[rbase:rbase + BS] = o
    return out


def emulate(inputs, pp=None):
    """Full numpy emulation; returns [1, OUT_C]."""
    if pp is None:
        pp = preprocess(np.asarray(inputs["edge_index"]))
    x = np.asarray(inputs["x"], np.float32)
    W0e, W1e, W2e = host_weights(inputs)
    h = np.zeros((RTOT, IN_C), np.float32)
    valid = pp["xperm"] >= 0
    h[valid] = x[pp["xperm"][valid]]

    b0r = np.tile(np.asarray(inputs["b0"], np.float32), (1, 1))
    b1r = np.tile(np.asarray(inputs["b1"], np.float32), (1, 1))
    b2r = np.tile(np.asarray(inputs["b2"], np.float32), (1, 1))

    h0 = _emulate_layer(h, pp, W0e, b0r, HEADS, F1, True, EL01)
    h1 = _emulate_layer(h0, pp, W1e, b1r, HEADS, F1, True, EL01)
    h2 = _emulate_layer(h1, pp, W2e, b2r, 1, HID, False, EL2)

    g = h2[valid].sum(axis=0, keepdims=True) / N
    return (g @ np.asarray(inputs["hw"], np.float32)
            + np.asarray(inputs["hb"], np.float32)).astype(np.float32)


# ---------------- device kernel ----------------

_BUILT = None


def build_kernel(upto=99, pp=None):
    import concourse.bacc as bacc
    import concourse.bass as bass
    import concourse.mybir as mybir
    import concourse.tile as tile
    from concourse import library_config

    f32 = mybir.dt.float32
    tb_dt = mybir.dt.bfloat16 if USE_BF16 else mybir.dt.float32
    i16 = mybir.dt.int16
    Alu = mybir.AluOpType
    Act = mybir.ActivationFunctionType

    nc = bacc.Bacc("TRN2", target_bir_lowering=False, debug=False,
                   num_devices=NCORES, num_swdge_queues=4)

    # ---- I/O ----
    xtb_d = nc.dram_tensor("xtb", [NB, IN_C, BS], f32, kind="ExternalInput")
    idx16_d = nc.dram_tensor("idx16", [128, NB * 2 * NKCOLS], i16,
                             kind="ExternalInput")
    dstc_d = nc.dram_tensor("dstc", [128, NB * 2 * TL], f32,
                            kind="ExternalInput")
    dstr_d = nc.dram_tensor("dstr", [128, KE], tb_dt, kind="ExternalInput")
    maskc_d = nc.dram_tensor("maskc", [128, NB], f32, kind="ExternalInput")
    w0e_d = nc.dram_tensor("w0e", [IN_C, F1 + 2 * HEADS], f32,
                           kind="ExternalInput")
    w1e_d = nc.dram_tensor("w1e", [2, 128, F1 + 2 * HEADS], f32,
                           kind="ExternalInput")
    w2e_d = nc.dram_tensor("w2e", [2, 128, HID + 2], f32,
                           kind="ExternalInput")
    b0r_d = nc.dram_tensor("b0r", [128, F1], f32, kind="ExternalInput")
    b1r_d = nc.dram_tensor("b1r", [128, F1], f32, kind="ExternalInput")
    b2r_d = nc.dram_tensor("b2r", [128, HID], f32, kind="ExternalInput")
    iota_row_d = nc.dram_tensor("iota_row", [128, 128], f32,
                                kind="ExternalInput")
    iota_col_d = nc.dram_tensor("iota_col", [128, 1], f32,
                                kind="ExternalInput")
    ones1_d = nc.dram_tensor("ones1", [1, 128], tb_dt, kind="ExternalInput")
    ident_d = nc.dram_tensor("ident", [128, 128], f32, kind="ExternalInput")
    out_d = nc.dram_tensor("out_part", [1, OUT_C], f32, kind="ExternalOutput")
    debug = os.environ.get("GAT_DEBUG", "0") == "1"
    if debug:
        dmp_tb = nc.dram_tensor("dmp_tb", [NPC, EL01], f32,
                                kind="ExternalOutput")
        dmp_h = nc.dram_tensor("dmp_h", [NPC, F1], f32, kind="ExternalOutput")
        dmp_den = nc.dram_tensor("dmp_den", [NPC, HEADS], f32,
                                 kind="ExternalOutput")
        dmp_tmp = nc.dram_tensor("dmp_tmp", [128, TL, F1], f32,
                                 kind="ExternalOutput")
        dmp_agg = nc.dram_tensor("dmp_agg", [128, F1 + HEADS], f32,
                                 kind="ExternalOutput")
        dmp_g = nc.dram_tensor("dmp_g", [128, TL, EL01], f32,
                               kind="ExternalOutput")
        dmp_s = nc.dram_tensor("dmp_s", [128, TL * HEADS], f32,
                               kind="ExternalOutput")

    # internal DRAM
    shared = os.environ.get("GAT_SHARED", "1") == "1"
    kw = dict(addr_space="Shared") if shared else {}
    shard01 = nc.dram_tensor("shard01", [NPC, EL01], tb_dt)
    table01 = nc.dram_tensor("table01", [RTOT, EL01], tb_dt, **kw)
    shard2 = nc.dram_tensor("shard2", [NPC, EL2], tb_dt)
    table2 = nc.dram_tensor("table2", [RTOT, EL2], tb_dt, **kw)

    rg = [list(range(NCORES))]

    with tile.TileContext(nc) as tc:
        with (
            tc.tile_pool(name="const", bufs=1) as cpool,
            tc.tile_pool(name="big", bufs=1) as bigpool,
            tc.tile_pool(name="work", bufs=3) as wpool,
            tc.tile_pool(name="gather", bufs=5) as gpool,
            tc.tile_pool(name="small", bufs=4) as spool,
            tc.tile_pool(name="psum", bufs=2, space="PSUM") as ppool,
            tc.tile_pool(name="psum1", bufs=1, space="PSUM") as ppool1,
        ):
            # ---- load constants ----
            def load_const(tag, dram, shape, dtype=f32, view=None):
                t = cpool.tile(shape, dtype, tag=tag)
                nc.sync.dma_start(out=t[:], in_=view if view is not None
                                  else dram[:])
                return t

            w0e_s = load_const("w0e", w0e_d, [IN_C, F1 + 2 * HEADS])
            w1e_s = load_const("w1e", w1e_d, [128, 2, F1 + 2 * HEADS],
                               view=w1e_d[:].rearrange("c p j -> p c j"))
            w2e_s = load_const("w2e", w2e_d, [128, 2, HID + 2],
                               view=w2e_d[:].rearrange("c p j -> p c j"))
            b0r_s = load_const("b0r", b0r_d, [128, F1])
            b1r_s = load_const("b1r", b1r_d, [128, F1])
            b2r_s = load_const("b2r", b2r_d, [128, HID])
            iota_row_s = load_const("iota_row", iota_row_d, [128, 128])
            iota_col_s = load_const("iota_col", iota_col_d, [128, 1])
            ones1_s = load_const("ones1", ones1_d, [1, 128], tb_dt)
            ident_s = load_const("ident", ident_d, [128, 128])
            idx16_s = load_const("idx16", idx16_d,
                                 [128, NB * 2 * NKCOLS], i16)
            dstc_s = load_const("dstc", dstc_d, [128, NB * 2 * TL])
            maskc_s = load_const("maskc", maskc_d, [128, NB])

            nc.gpsimd.load_library(library_config.mlp)

            hT = bigpool.tile([128, 2, NPC], f32, tag="hT")

            def transform(layer):
                """Own-shard transform -> shard DRAM + ad_all SBUF."""
                heads = 1 if layer == 2 else HEADS
                Fo = HID if layer == 2 else F1
                ncols = Fo + 2 * heads
                el = EL2 if layer == 2 else EL01
                shard = shard2 if layer == 2 else shard01
                ad_all = spool.tile([128, NB * heads], tb_dt, tag="ad_all")
                for b in range(NB):
                    ps = ppool.tile([128, 512], f32, tag="agg", space="PSUM")
                    if layer == 0:
                        xb = wpool.tile([IN_C, BS], f32, tag="xtb")
                        nc.sync.dma_start(out=xb[:], in_=xtb_d[b])
                        nc.tensor.matmul(out=ps[:, :ncols], lhsT=xb[:],
                                         rhs=w0e_s[:], start=True, stop=True)
                    else:
                        we = w1e_s if layer == 1 else w2e_s
                        for k2 in range(2):
                            nc.tensor.matmul(
                                out=ps[:, :ncols],
                                lhsT=hT[:, k2, b * BS:(b + 1) * BS],
                                rhs=we[:, k2, :],
                                start=(k2 == 0), stop=(k2 == 1))
                    tb = wpool.tile([128, el], tb_dt, tag="tbout")
                    nc.vector.tensor_copy(out=tb[:, :ncols],
                                          in_=ps[:, :ncols])
                    nc.vector.tensor_copy(
                        out=ad_all[:, b * heads:(b + 1) * heads],
                        in_=ps[:, Fo + heads:Fo + 2 * heads])
                    nc.sync.dma_start(out=shard[b * BS:(b + 1) * BS, :],
                                      in_=tb[:])
                    if debug and layer == 0:
                        nc.sync.dma_start(
                            out=dmp_tb[b * BS:(b + 1) * BS, :], in_=tb[:])
                return ad_all

            def allgather(layer):
                shard = shard2 if layer == 2 else shard01
                table = table2 if layer == 2 else table01
                nc.gpsimd.collective_compute(
                    "AllGather", mybir.AluOpType.bypass,
                    replica_groups=rg, ins=[shard[:].opt()],
                    outs=[table[:].opt()])

            def aggregate(layer, ad_all):
                sub = int(os.environ.get("GAT_AGG_SUB", "99"))
                heads = 1 if layer == 2 else HEADS
                Fo = HID if layer == 2 else F1
                el = EL2 if layer == 2 else EL01
                table = table2 if layer == 2 else table01
                brep = (b2r_s, b1r_s, b1r_s)[0] if False else (
                    b0r_s if layer == 0 else (b1r_s if layer == 1 else b2r_s))
                views = [table[0:LO_LIM, :], table[HI_BASE:HI_BASE + 32768, :]]
                if layer == 2:
                    psum_sum = ppool1.tile([1, OUT_C], f32, tag="sum",
                                          space="PSUM")
                for b in range(NB):
                    pagg = ppool.tile([128, Fo], f32, tag="agg",
                                      space="PSUM")
                    pden = ppool.tile([128, heads], f32, tag="den_ps",
                                      space="PSUM")
                    for kind in range(2):
                        bk = b * 2 + kind
                        g = gpool.tile([128, TL, el], tb_dt, tag="g")
                        for i3 in range(3):
                            nc.gpsimd.dma_gather(
                                g[:, 3 * i3:3 * (i3 + 1), :], views[kind],
                                idx16_s[:, bk * NKCOLS + 24 * i3:
                                        bk * NKCOLS + 24 * (i3 + 1)],
                                384, 384, el, single_packet=False,
                                queue_num=(3 * bk + i3) % 4)
                        if sub < 2:
                            continue
                        # one-hot M [128e, TL*128d]
                        M = wpool.tile([128, KE], tb_dt, tag="M")
                        tcol = b * 2 * TL + kind * TL
                        nc.vector.tensor_tensor(
                            out=M[:].rearrange("p (t d) -> p t d", t=TL),
                            in0=dstc_s[:, tcol:tcol + TL].unsqueeze(-1)
                                .broadcast_to([128, TL, 128]),
                            in1=iota_row_s[:].unsqueeze(1)
                                .broadcast_to([128, TL, 128]),
                            op=Alu.is_equal)
                        if sub < 3:
                            continue
                        # M_T [128d, TL*128e] via replicated-row outer product
                        MT = wpool.tile([128, KE], tb_dt, tag="MT")
                        dr = spool.tile([1, KE], tb_dt, tag="dr")
                        nc.sync.dma_start(out=dr[:], in_=dstr_d[bk:bk + 1, :])
                        for o, wdt in ((0, 512), (512, 512), (1024, 128)):
                            pr = ppool1.tile([128, 512], f32, tag="rep",
                                            space="PSUM")
                            nc.tensor.matmul(out=pr[:, :wdt],
                                             lhsT=ones1_s[:],
                                             rhs=dr[:, o:o + wdt],
                                             start=True, stop=True)
                            nc.vector.tensor_tensor(
                                out=MT[:, o:o + wdt], in0=pr[:, :wdt],
                                in1=iota_col_s[:]
                                    .broadcast_to([128, wdt]),
                                op=Alu.is_equal)
                        if sub < 4:
                            continue
                        # ad per edge via M_T @ ad_block
                        pad_ = ppool1.tile([128, TL * heads], f32, tag="adp",
                                          space="PSUM")
                        for t in range(TL):
                            nc.tensor.matmul(
                                out=pad_[:, t * heads:(t + 1) * heads],
                                lhsT=MT[:, t * 128:(t + 1) * 128],
                                rhs=ad_all[:, b * heads:(b + 1) * heads],
                                start=True, stop=True)
                        if sub < 5:
                            continue
                        # z = as + ad ; s = exp(max(z, 0.2 z))
                        z = spool.tile([128, TL * heads], f32, tag="z")
                        nc.vector.tensor_tensor(
                            out=z[:].rearrange("p (t h) -> p t h", t=TL),
                            in0=g[:, :, Fo:Fo + heads],
                            in1=pad_[:].rearrange("p (t h) -> p t h", t=TL),
                            op=Alu.add)
                        zm = spool.tile([128, TL * heads], f32, tag="zm")
                        nc.scalar.activation(zm[:], z[:], Act.Prelu,
                                             alpha=0.2)
                        s_t = spool.tile([128, TL * heads], tb_dt, tag="s")
                        nc.scalar.activation(s_t[:], zm[:], Act.Exp)
                        if sub < 6:
                            continue
                        # tmp = g[:, :, :Fo] * s (broadcast over HID),
                        # one 3D op per head (4D broadcast APs miscompute)
                        tmp = wpool.tile([128, TL, Fo], tb_dt, tag="tmp")
                        sv = s_t[:].rearrange("p (t h) -> p t h", t=TL)
                        for hh in range(heads):
                            nc.vector.tensor_tensor(
                                out=tmp[:, :, hh * HID:(hh + 1) * HID],
                                in0=g[:, :, hh * HID:(hh + 1) * HID],
                                in1=sv[:, :, hh:hh + 1]
                                    .broadcast_to([128, TL, HID]),
                                op=Alu.mult)
                        if debug and layer == 0 and b == 0 and kind == 0:
                            nc.sync.dma_start(out=dmp_tmp[:], in_=tmp[:])
                            nc.sync.dma_start(out=dmp_g[:], in_=g[:])
                            nc.sync.dma_start(out=dmp_s[:], in_=s_t[:])
                        if sub < 7:
                            continue
                        # accumulate
                        for t in range(TL):
                            first = (kind == 0 and t == 0)
                            last = (kind == 1 and t == TL - 1)
                            nc.tensor.matmul(
                                out=pagg[:],
                                lhsT=M[:, t * 128:(t + 1) * 128],
                                rhs=tmp[:, t, :],
                                start=first, stop=last)
                            nc.tensor.matmul(
                                out=pden[:],
                                lhsT=M[:, t * 128:(t + 1) * 128],
                                rhs=s_t[:, t * heads:(t + 1) * heads],
                                start=first, stop=last)
                    if sub < 8:
                        continue
                    # epilogue
                    if debug and layer == 0 and b == 0:
                        aggc = wpool.tile([128, F1 + HEADS], f32, tag="aggc")
                        nc.vector.tensor_copy(out=aggc[:, :F1], in_=pagg[:])
                        nc.vector.tensor_copy(out=aggc[:, F1:], in_=pden[:])
                        nc.sync.dma_start(out=dmp_agg[:], in_=aggc[:])
                    den = spool.tile([128, heads], f32, tag="den")
                    nc.vector.tensor_scalar(out=den[:],
                                            in0=pden[:],
                                            scalar1=1e-16, scalar2=None,
                                            op0=Alu.add)
                    rec = spool.tile([128, heads], f32, tag="rec")
                    nc.vector.reciprocal(out=rec[:], in_=den[:])
                    if debug and layer == 0:
                        nc.sync.dma_start(
                            out=dmp_den[b * BS:(b + 1) * BS, :], in_=den[:])
                    o1 = wpool.tile([128, Fo], f32, tag="o1")
                    nc.vector.tensor_tensor(
                        out=o1[:].rearrange("p (h f) -> p h f", h=heads),
                        in0=pagg[:].rearrange("p (h f) -> p h f",
                                              h=heads),
                        in1=rec[:].unsqueeze(-1)
                            .broadcast_to([128, heads, HID]),
                        op=Alu.mult)
                    o2 = wpool.tile([128, Fo], f32, tag="o2")
                    nc.vector.tensor_tensor(out=o2[:], in0=o1[:],
                                            in1=brep[:, :Fo], op=Alu.add)
                    if layer == 2:
                        nc.tensor.matmul(out=psum_sum[:],
                                         lhsT=maskc_s[:, b:b + 1],
                                         rhs=o2[:], start=(b == 0),
                                         stop=(b == NB - 1))
                    else:
                        o3 = wpool.tile([128, Fo], f32, tag="o3")
                        nc.scalar.activation(o3[:], o2[:], Act.Relu)
                        if debug and layer == 0:
                            nc.sync.dma_start(
                                out=dmp_h[b * BS:(b + 1) * BS, :], in_=o3[:])
                        for k2 in range(2):
                            pt = ppool1.tile([128, 128], f32, tag="tp",
                                            space="PSUM")
                            nc.tensor.transpose(
                                pt[:], o3[:, k2 * 128:(k2 + 1) * 128],
                                ident_s[:])
                            nc.vector.tensor_copy(
                                out=hT[:, k2, b * BS:(b + 1) * BS],
                                in_=pt[:])
                if layer == 2:
                    osb = spool.tile([1, OUT_C], f32, tag="osb")
                    nc.vector.tensor_copy(out=osb[:], in_=psum_sum[:])
                    nc.sync.dma_start(out=out_d[:], in_=osb[:])

            stage = 0
            for layer in range(3):
                if stage >= upto:
                    break
                ad_all = transform(layer)
                stage += 1
                if stage >= upto:
                    break
                allgather(layer)
                stage += 1
                if stage >= upto:
                    break
                aggregate(layer, ad_all)
                stage += 1

    nc.compile()
    return nc


def _get_built(pp=None):
    global _BUILT
    if _BUILT is None:
        _BUILT = build_kernel(upto=int(os.environ.get("GAT_UPTO", "99")),
                              pp=pp)
    return _BUILT


def kernel(**inputs) -> np.ndarray:
    from concourse.bass_utils import run_bass_kernel_spmd

    pp = preprocess(np.asarray(inputs["edge_index"]))
    in_maps = build_core_inputs(inputs, pp)
    nc = _get_built()
    res = run_bass_kernel_spmd(nc, in_maps, core_ids=list(range(NCORES)))
    parts = np.stack([r["out_part"][0] for r in res.results])  # [8, 64]
    g = parts.sum(axis=0, keepdims=True) / N
    out = (g @ np.asarray(inputs["hw"], np.float32)
           + np.asarray(inputs["hb"], np.float32)).astype(np.float32)
    return out

